# revision 26
# baseline (speedup 1.0000x reference)
"""Trainium2 Bass kernel for nn_GCNBackbone (3-layer GNN + attention pooling).

Self-contained: accepts FULL inputs, shards across 8 NeuronCores by dst-node
blocks (whole graphs per core), runs a Bass/Tile SPMD program, gathers the
full output.

Key algebra: msg = concat(h[src], ea) @ W + b summed over incoming edges
  == (sum h[src]) @ W_h + (sum ea) @ W_e + deg * b
so the per-edge matmul collapses to per-node matmuls; the irregular part is
a gather of h rows plus a segmented sum done as one-hot matmuls on PE.
"""
import numpy as np
import ml_dtypes

import concourse.bacc as bacc
import concourse.bass as bass
import concourse.tile as tile
from concourse import mybir
from concourse.bass_utils import run_bass_kernel_spmd
from concourse.masks import make_identity

F32 = mybir.dt.float32
BF16 = mybir.dt.bfloat16
I32 = mybir.dt.int32
AF = mybir.ActivationFunctionType
OP = mybir.AluOpType

B, T, K = 32, 16, 128
NUM_TYPES = 4
N = B * T * K
E_GLOBAL = 262144
IN_DIM, EDGE_DIM, TIME_DIM = 64, 32, 32
ENC_DIM, HID = 224, 256
NUM_LAYERS = 3
NCORES = 8
NPC = N // NCORES
NBLK = NPC // 128
P = 128
EPS = 1e-5

_PROG_CACHE = {}


def _bcast_rows(ap, p=P):
    """DRAM row AP -> partition-broadcast AP [[0,p], free...]."""
    return bass.AP(tensor=ap.tensor, offset=ap.offset, ap=[[0, p]] + list(ap.ap[1:]))


def _prep(x, edge_attr, time_emb, params, edge_index, batch_idx, temporal, node_type):
    x = np.ascontiguousarray(np.asarray(x, np.float32))
    edge_attr = np.ascontiguousarray(np.asarray(edge_attr, np.float32))
    time_emb = np.ascontiguousarray(np.asarray(time_emb, np.float32))
    ei = np.asarray(edge_index, np.int64)
    src, dst = ei[0], ei[1]

    node = np.arange(N, dtype=np.int64)
    assert np.array_equal(np.asarray(batch_idx, np.int64), node // (T * K))
    assert np.array_equal(np.asarray(temporal, np.int64), (node // K) % T)
    assert np.array_equal(np.asarray(node_type, np.int64), node % NUM_TYPES)

    core_of = dst // NPC
    blk_of = (dst % NPC) // P
    cnt = np.zeros((NCORES, NBLK), np.int64)
    for c in range(NCORES):
        cnt[c] = np.bincount(blk_of[core_of == c], minlength=NBLK)
    T_blk = np.maximum(1, -(-cnt // P)).max(axis=0)       # per-block max over cores
    T_tot = int(T_blk.sum())
    tile_start = np.zeros(NBLK + 1, np.int64)
    tile_start[1:] = np.cumsum(T_blk)

    per_core = []
    for c in range(NCORES):
        m = np.nonzero(core_of == c)[0]
        order = m[np.argsort(blk_of[m], kind="stable")]
        gsrc = np.zeros((P, T_tot), np.int32)
        onehot = np.zeros((P, T_tot, P), np.float32)
        ea_aug = np.zeros((T_tot * P, EDGE_DIM + 1), np.float32)
        pos = 0
        for b in range(NBLK):
            nb = int(cnt[c, b])
            eb = order[pos:pos + nb]
            pos += nb
            if nb:
                dloc = ((dst[eb] % NPC) % P).astype(np.int64)
                i = np.arange(nb)
                tt = tile_start[b] + i // P
                pp = i % P
                sc = src[eb] // NPC
                sr = src[eb] % NPC
                half = NPC // 2
                gsrc[pp, tt] = np.where(
                    sr < half, sc * half + sr,
                    N // 2 + sc * half + (sr - half)).astype(np.int32)
                onehot[pp, tt, dloc] = 1.0
                ea_aug[tt * P + pp, :EDGE_DIM] = edge_attr[eb]
                ea_aug[tt * P + pp, EDGE_DIM] = 1.0
        assert pos == len(order)
        xc = x[c * NPC:(c + 1) * NPC]
        xT_aug = np.ones((IN_DIM + 1, NPC), np.float32)
        xT_aug[:IN_DIM] = xc.T
        g_ids = (c * NPC + np.arange(NBLK) * P) // (T * K)
        time_rows = time_emb[g_ids]                        # [64, 32]
        per_core.append(dict(
            gsrc=gsrc,
            onehot=np.ascontiguousarray(onehot.reshape(P, T_tot * P)).astype(ml_dtypes.bfloat16),
            ea_aug=ea_aug,
            xT_aug=xT_aug,
            time_rows=np.ascontiguousarray(time_rows),
        ))

    p_ = {k: np.asarray(v, np.float32) for k, v in params.items()}
    Ws = [p_["W_c0"], p_["W_c1"], p_["W_c2"]]
    bs = [p_["b_c0"], p_["b_c1"], p_["b_c2"]]
    tm = np.zeros((P, NUM_TYPES), np.float32)
    tm[np.arange(P), np.arange(P) % NUM_TYPES] = 1.0
    sm = np.zeros((P, 32), np.float32)
    sm[np.arange(P), np.arange(P) // 4] = 1.0
    weights = dict(
        W1_aug=np.ascontiguousarray(np.vstack([p_["W_enc1"], p_["b_enc1"][None, :]])),
        W_enc2=p_["W_enc2"],
        genc_col=np.ascontiguousarray(p_["g_enc"][:, None]),
        benc_col=np.ascontiguousarray(p_["be_enc"][:, None]),
        b_enc2_row=np.ascontiguousarray(p_["b_enc2"][None, :]),
        W_h_all=np.ascontiguousarray(np.stack([Ws[i][:HID] for i in range(3)])),
        W_e_all=np.ascontiguousarray(
            np.stack([np.vstack([Ws[i][HID:], bs[i][None, :]]) for i in range(3)])),
        ln_g_all=np.ascontiguousarray(p_["ln_g"]),
        ln_b_all=np.ascontiguousarray(p_["ln_b"]),
        W_a1=p_["W_a1"],
        b_a1_row=np.ascontiguousarray(p_["b_a1"][None, :]),
        W_a2_col=np.ascontiguousarray(p_["W_a2"]),
        b_a2=np.ascontiguousarray(p_["b_a2"][None, :]),      # [1,1]
        W_g_row=np.ascontiguousarray(p_["W_g"].T),           # [1,256]
        b_g=np.ascontiguousarray(p_["b_g"][None, :]),        # [1,1]
        typemask=tm.astype(ml_dtypes.bfloat16),
        summat=sm.astype(ml_dtypes.bfloat16),
    )
    return per_core, weights, T_blk, T_tot, tile_start


def _build(T_blk, T_tot, tile_start):
    nc = bacc.Bacc(None, target_bir_lowering=False, num_devices=NCORES,
                   dynamic_dma_scratch_size=32768)

    dp = nc.declare_dram_parameter
    xT_aug = dp("xT_aug", [IN_DIM + 1, NPC], F32, isOutput=False)
    gsrc = dp("gsrc", [P, T_tot], I32, isOutput=False)
    onehot = dp("onehot", [P, T_tot * P], BF16, isOutput=False)
    ea_aug = dp("ea_aug", [T_tot * P, EDGE_DIM + 1], F32, isOutput=False)
    time_rows = dp("time_rows", [NBLK, TIME_DIM], F32, isOutput=False)
    W1_aug = dp("W1_aug", [IN_DIM + 1, HID], F32, isOutput=False)
    W_enc2 = dp("W_enc2", [HID, ENC_DIM], F32, isOutput=False)
    genc_col = dp("genc_col", [HID, 1], F32, isOutput=False)
    benc_col = dp("benc_col", [HID, 1], F32, isOutput=False)
    b_enc2_row = dp("b_enc2_row", [1, ENC_DIM], F32, isOutput=False)
    W_h_all = dp("W_h_all", [3, HID, HID], F32, isOutput=False)
    W_e_all = dp("W_e_all", [3, EDGE_DIM + 1, HID], F32, isOutput=False)
    ln_g_all = dp("ln_g_all", [3, HID], F32, isOutput=False)
    ln_b_all = dp("ln_b_all", [3, HID], F32, isOutput=False)
    W_a1 = dp("W_a1", [HID, P], F32, isOutput=False)
    b_a1_row = dp("b_a1_row", [1, P], F32, isOutput=False)
    W_a2_col = dp("W_a2_col", [P, 1], F32, isOutput=False)
    b_a2_in = dp("b_a2", [1, 1], F32, isOutput=False)
    W_g_row = dp("W_g_row", [1, HID], F32, isOutput=False)
    b_g_in = dp("b_g", [1, 1], F32, isOutput=False)
    typemask_in = dp("typemask", [P, NUM_TYPES], BF16, isOutput=False)
    summat_in = dp("summat", [P, 32], BF16, isOutput=False)
    out_dram = dp("out", [NBLK, HID], F32, isOutput=True)

    # bf16 shadows in DRAM (SWDGE cast once, HWDGE loads after)
    xT_bf = nc.dram_tensor("xT_bf", [IN_DIM + 1, NPC], BF16)
    ea_bf = nc.dram_tensor("ea_bf", [T_tot * P, EDGE_DIM + 1], BF16)
    W1_bf = nc.dram_tensor("W1_bf", [IN_DIM + 1, HID], BF16)
    Wh_bf = nc.dram_tensor("Wh_bf", [3, HID, HID], BF16)
    We_bf = nc.dram_tensor("We_bf", [3, EDGE_DIM + 1, HID], BF16)
    Wa1_bf = nc.dram_tensor("Wa1_bf", [HID, P], BF16)
    ba1_bf_d = nc.dram_tensor("ba1_bf", [1, P], BF16)
    Wa2_bf_d = nc.dram_tensor("Wa2_bf", [P, 1], BF16)
    ba2_bf_d = nc.dram_tensor("ba2_bf", [1, 1], BF16)
    tmr_bf_d = nc.dram_tensor("tmr_bf", [1, NBLK * TIME_DIM], BF16)

    h_locA = [nc.dram_tensor(f"h_locA{i}", [NPC // 2, HID], BF16) for i in range(3)]
    h_locB = [nc.dram_tensor(f"h_locB{i}", [NPC // 2, HID], BF16) for i in range(3)]
    h_all = [nc.dram_tensor(f"h_all{i}", [N, HID], BF16, addr_space="Shared")
             for i in range(3)]
    groups = [list(range(NCORES))]

    with tile.TileContext(nc) as tc:
        with (
            tc.tile_pool(name="big", bufs=1) as big,
            tc.tile_pool(name="acc", bufs=1) as accp,
            tc.tile_pool(name="gath", bufs=16) as gp,
            tc.tile_pool(name="work", bufs=4) as wk,
            tc.tile_pool(name="sm", bufs=8) as smp,
            tc.tile_pool(name="pp2", bufs=2, space="PSUM") as pp2,
            tc.tile_pool(name="pp1", bufs=2, space="PSUM") as pp1,
        ):
            # ---------- one-time casts (SWDGE) ----------
            nc.gpsimd.dma_start(out=xT_bf[:], in_=xT_aug[:])
            nc.gpsimd.dma_start(out=ea_bf[:], in_=ea_aug[:])
            nc.gpsimd.dma_start(out=W1_bf[:], in_=W1_aug[:])
            nc.gpsimd.dma_start(out=Wh_bf[:], in_=W_h_all[:])
            nc.gpsimd.dma_start(out=We_bf[:], in_=W_e_all[:])
            nc.gpsimd.dma_start(out=Wa1_bf[:], in_=W_a1[:])
            nc.gpsimd.dma_start(out=ba1_bf_d[:], in_=b_a1_row[:])
            nc.gpsimd.dma_start(out=Wa2_bf_d[:], in_=W_a2_col[:])
            nc.gpsimd.dma_start(out=ba2_bf_d[:], in_=b_a2_in[:])
            nc.gpsimd.dma_start(out=tmr_bf_d[:],
                                in_=time_rows[:].rearrange("b t -> (b t)")[None, :])

            # ---------- resident SBUF state ----------
            onehot_sb = big.tile([P, T_tot * P], BF16)
            nc.sync.dma_start(out=onehot_sb[:], in_=onehot[:])
            gsrc_sb = big.tile([P, T_tot], I32)
            nc.sync.dma_start(out=gsrc_sb[:], in_=gsrc[:])
            aggTe_sb = big.tile([EDGE_DIM + 1, NPC], BF16)
            W1_sb = big.tile([IN_DIM + 1, HID], BF16)
            nc.sync.dma_start(out=W1_sb[:], in_=W1_bf[:])
            Wh_sb = big.tile([P, 3, 2, HID], BF16)
            nc.sync.dma_start(out=Wh_sb[:],
                              in_=Wh_bf[:].rearrange("l (c p) n -> p l c n", p=P))
            We_sb = big.tile([EDGE_DIM + 1, 3, HID], BF16)
            nc.sync.dma_start(out=We_sb[:],
                              in_=We_bf[:].rearrange("l e n -> e l n"))
            Wa1_sb = big.tile([P, 2, P], BF16)
            nc.sync.dma_start(out=Wa1_sb[:],
                              in_=Wa1_bf[:].rearrange("(c p) a -> p c a", p=P))
            ba1_sb = big.tile([1, P], BF16)
            nc.sync.dma_start(out=ba1_sb[:], in_=ba1_bf_d[:])
            Wa2_sb = big.tile([P, 1], BF16)
            nc.sync.dma_start(out=Wa2_sb[:], in_=Wa2_bf_d[:])
            ba2_sb = big.tile([1, 1], BF16)
            nc.sync.dma_start(out=ba2_sb[:], in_=ba2_bf_d[:])
            time_sb = big.tile([1, NBLK * TIME_DIM], BF16)
            nc.sync.dma_start(out=time_sb[:], in_=tmr_bf_d[:])
            typemask_sb = big.tile([P, NUM_TYPES], BF16)
            nc.sync.dma_start(out=typemask_sb[:], in_=typemask_in[:])
            summat_sb = big.tile([P, 32], BF16)
            nc.sync.dma_start(out=summat_sb[:], in_=summat_in[:])
            # broadcast loads (SWDGE cast f32->bf16, partition step 0)
            lng_sb = big.tile([P, 3, HID], BF16)
            lnb_sb = big.tile([P, 3, HID], BF16)
            for i in range(3):
                nc.gpsimd.dma_start(out=lng_sb[:, i, :], in_=_bcast_rows(ln_g_all[i:i + 1, :]))
                nc.gpsimd.dma_start(out=lnb_sb[:, i, :], in_=_bcast_rows(ln_b_all[i:i + 1, :]))
            wg_sb = big.tile([P, HID], BF16)
            nc.gpsimd.dma_start(out=wg_sb[:], in_=_bcast_rows(W_g_row[:]))
            bg_sb = big.tile([P, 1], F32)
            nc.gpsimd.dma_start(out=bg_sb[:], in_=_bcast_rows(b_g_in[:]))

            eps_sb = big.tile([P, 1], F32)
            nc.vector.memset(eps_sb[:], EPS)
            ones_bf = big.tile([1, P], BF16)
            nc.vector.memset(ones_bf[:], 1.0)
            ones11_bf = big.tile([1, 1], BF16)
            nc.vector.memset(ones11_bf[:], 1.0)
            ones11_f = big.tile([1, 1], F32)
            nc.vector.memset(ones11_f[:], 1.0)
            ident = big.tile([P, P], BF16)
            make_identity(nc, ident[:])
            identf = big.tile([P, P], F32)
            make_identity(nc, identf[:])

            # ---------- encoder weight folds (device, f32) ----------
            W2_sb = big.tile([P, 2, ENC_DIM], F32)
            nc.sync.dma_start(out=W2_sb[:], in_=W_enc2[:].rearrange("(c p) n -> p c n", p=P))
            b2r_sb = big.tile([1, ENC_DIM], F32)
            nc.sync.dma_start(out=b2r_sb[:], in_=b_enc2_row[:])
            W2p_sb = big.tile([P, 2, ENC_DIM], BF16)
            b2p_sb = big.tile([1, ENC_DIM], BF16)
            ps_b2 = pp1.tile([1, ENC_DIM], F32, tag="pe")
            for k2 in range(2):
                gch = smp.tile([P, 1], F32, tag="gch")
                nc.sync.dma_start(out=gch[:], in_=genc_col[k2 * P:(k2 + 1) * P, :])
                nc.vector.tensor_scalar(out=W2p_sb[:, k2, :], in0=W2_sb[:, k2, :],
                                        scalar1=gch[:], scalar2=None, op0=OP.mult)
                bch = smp.tile([P, 1], F32, tag="bch")
                nc.sync.dma_start(out=bch[:], in_=benc_col[k2 * P:(k2 + 1) * P, :])
                nc.tensor.matmul(ps_b2[:], lhsT=bch[:], rhs=W2_sb[:, k2, :],
                                 start=(k2 == 0), stop=False)
            nc.tensor.matmul(ps_b2[:], lhsT=ones11_f[:], rhs=b2r_sb[:],
                             start=False, stop=True)
            nc.scalar.copy(b2p_sb[:], ps_b2[:])

            accs = [accp.tile([P, HID], BF16, tag=f"a{b}", name=f"acc{b}")
                    for b in range(NBLK)]

            # ---------- encoder ----------
            for b in range(NBLK):
                xt = wk.tile([IN_DIM + 1, P], BF16, tag="xt")
                nc.sync.dma_start(out=xt[:], in_=xT_bf[:, b * P:(b + 1) * P])
                ps_h1 = pp2.tile([P, HID], F32, tag="pa1")
                nc.tensor.matmul(ps_h1[:], lhsT=xt[:], rhs=W1_sb[:], start=True, stop=True)
                h1 = wk.tile([P, HID], F32, tag="h1")
                nc.scalar.activation(out=h1[:], in_=ps_h1[:], func=AF.Relu)
                stats = smp.tile([P, 6], F32, tag="st")
                nc.vector.bn_stats(out=stats[:], in_=h1[:])
                mv = smp.tile([P, 2], F32, tag="mv")
                nc.vector.bn_aggr(out=mv[:], in_=stats[:])
                rstd = smp.tile([P, 1], F32, tag="rstd")
                nc.scalar.activation(out=rstd[:], in_=mv[:, 1:2], func=AF.Sqrt,
                                     bias=eps_sb[:], scale=1.0)
                nc.vector.reciprocal(rstd[:], rstd[:])
                nm = smp.tile([P, 1], F32, tag="nm")
                nc.vector.tensor_scalar(out=nm[:], in0=mv[:, 0:1], scalar1=rstd[:],
                                        scalar2=-1.0, op0=OP.mult, op1=OP.mult)
                h1n0 = wk.tile([P, HID], BF16, tag="h1n0")
                nc.vector.tensor_scalar(out=h1n0[:], in0=h1[:], scalar1=rstd[:],
                                        scalar2=nm[:], op0=OP.mult, op1=OP.add)
                ps_tt = pp2.tile([P, HID], BF16, tag="pa0")
                nc.tensor.transpose(ps_tt[:, 0:P], h1n0[:, 0:P], ident[:])
                nc.tensor.transpose(ps_tt[:, P:HID], h1n0[:, P:HID], ident[:])
                tT = wk.tile([P, HID], BF16, tag="tT")
                nc.scalar.copy(tT[:], ps_tt[:])
                ps_he = pp2.tile([P, HID], F32, tag="pz")
                nc.tensor.matmul(ps_he[:, :ENC_DIM], lhsT=tT[:, 0:P], rhs=W2p_sb[:, 0, :],
                                 start=True, stop=False)
                nc.tensor.matmul(ps_he[:, :ENC_DIM], lhsT=tT[:, P:HID], rhs=W2p_sb[:, 1, :],
                                 start=False, stop=False)
                nc.tensor.matmul(ps_he[:, :ENC_DIM], lhsT=ones_bf[:], rhs=b2p_sb[:],
                                 start=False, stop=True)
                nc.tensor.matmul(ps_he[:, ENC_DIM:], lhsT=ones_bf[:],
                                 rhs=time_sb[:, b * TIME_DIM:(b + 1) * TIME_DIM],
                                 start=True, stop=True)
                henc = wk.tile([P, HID], BF16, tag="henc")
                nc.vector.tensor_copy(henc[:], ps_he[:])
                if b < NBLK // 2:
                    nc.sync.dma_start(out=h_locA[0][b * P:(b + 1) * P, :], in_=henc[:])
                else:
                    nc.sync.dma_start(
                        out=h_locB[0][(b - NBLK // 2) * P:(b - NBLK // 2 + 1) * P, :],
                        in_=henc[:])
                if b == NBLK // 2 - 1:
                    nc.gpsimd.collective_compute(
                        "AllGather", OP.bypass, replica_groups=groups,
                        ins=[h_locA[0][:]], outs=[h_all[0][0:N // 2, :]])

            # ---------- layers ----------
            score_all = big.tile([P, NBLK], F32)
            for i in range(NUM_LAYERS):
                nc.gpsimd.collective_compute(
                    "AllGather", OP.bypass, replica_groups=groups,
                    ins=[h_locB[i][:]], outs=[h_all[i][N // 2:N, :]])
                if i == 0:
                    # aggTe pass (layer-independent) — fills the AG idle window
                    for b in range(NBLK):
                        ps_e = pp1.tile([EDGE_DIM + 1, P], F32, tag="pe")
                        t0, t1 = int(tile_start[b]), int(tile_start[b + 1])
                        for t in range(t0, t1):
                            ea_t = wk.tile([P, EDGE_DIM + 1], BF16, tag="ea")
                            nc.sync.dma_start(out=ea_t[:], in_=ea_bf[t * P:(t + 1) * P, :])
                            nc.tensor.matmul(ps_e[:], lhsT=ea_t[:],
                                             rhs=onehot_sb[:, t * P:(t + 1) * P],
                                             start=(t == t0), stop=(t == t1 - 1))
                        nc.vector.tensor_copy(aggTe_sb[:, b * P:(b + 1) * P], ps_e[:])

                for b in range(NBLK):
                    t0, t1 = int(tile_start[b]), int(tile_start[b + 1])
                    ps_a0 = pp2.tile([P, P], F32, tag="pa0")
                    ps_a1 = pp2.tile([P, P], F32, tag="pa1")
                    for t in range(t0, t1):
                        g = gp.tile([P, HID], BF16, tag="g")
                        nc.gpsimd.indirect_dma_start(
                            out=g[:], out_offset=None, in_=h_all[i][:],
                            in_offset=bass.IndirectOffsetOnAxis(
                                ap=gsrc_sb[:, t:t + 1], axis=0))
                        oh = onehot_sb[:, t * P:(t + 1) * P]
                        nc.tensor.matmul(ps_a0[:], lhsT=g[:, 0:P], rhs=oh,
                                         start=(t == t0), stop=(t == t1 - 1))
                        nc.tensor.matmul(ps_a1[:], lhsT=g[:, P:HID], rhs=oh,
                                         start=(t == t0), stop=(t == t1 - 1))
                    a0 = wk.tile([P, HID], BF16, tag="a0")
                    nc.vector.tensor_copy(a0[:, 0:P], ps_a0[:])
                    nc.vector.tensor_copy(a0[:, P:HID], ps_a1[:])
                    ps_z = pp2.tile([P, HID], F32, tag="pz")
                    nc.tensor.matmul(ps_z[:], lhsT=a0[:, 0:P], rhs=Wh_sb[:, i, 0, :],
                                     start=True, stop=False)
                    nc.tensor.matmul(ps_z[:], lhsT=a0[:, P:HID], rhs=Wh_sb[:, i, 1, :],
                                     start=False, stop=False)
                    nc.tensor.matmul(ps_z[:], lhsT=aggTe_sb[:, b * P:(b + 1) * P],
                                     rhs=We_sb[:, i, :], start=False, stop=True)
                    stats = smp.tile([P, 6], F32, tag="st")
                    nc.vector.bn_stats(out=stats[:], in_=ps_z[:])
                    mv = smp.tile([P, 2], F32, tag="mv")
                    nc.vector.bn_aggr(out=mv[:], in_=stats[:])
                    rstd = smp.tile([P, 1], F32, tag="rstd")
                    nc.scalar.activation(out=rstd[:], in_=mv[:, 1:2], func=AF.Sqrt,
                                         bias=eps_sb[:], scale=1.0)
                    nc.vector.reciprocal(rstd[:], rstd[:])
                    nm = smp.tile([P, 1], F32, tag="nm")
                    nc.vector.tensor_scalar(out=nm[:], in0=mv[:, 0:1], scalar1=rstd[:],
                                            scalar2=-1.0, op0=OP.mult, op1=OP.mult)
                    tnorm = wk.tile([P, HID], BF16, tag="tn")
                    nc.scalar.activation(out=tnorm[:], in_=ps_z[:], func=AF.Identity,
                                         bias=nm[:], scale=rstd[:])
                    u = wk.tile([P, HID], BF16, tag="u")
                    nc.vector.tensor_tensor(out=u[:], in0=tnorm[:], in1=lng_sb[:, i, :],
                                            op=OP.mult)
                    v = wk.tile([P, HID], BF16, tag="v")
                    nc.vector.tensor_tensor(out=v[:], in0=u[:], in1=lnb_sb[:, i, :],
                                            op=OP.add)
                    if i == 0:
                        nc.scalar.activation(out=accs[b][:], in_=v[:], func=AF.Silu)
                    else:
                        hn = wk.tile([P, HID], BF16, tag="hn")
                        nc.scalar.activation(out=hn[:], in_=v[:], func=AF.Silu)
                        nc.vector.tensor_tensor(out=accs[b][:], in0=accs[b][:],
                                                in1=hn[:], op=OP.add)
                    if i < 2:
                        if b < NBLK // 2:
                            nc.sync.dma_start(out=h_locA[i + 1][b * P:(b + 1) * P, :],
                                              in_=accs[b][:])
                        else:
                            nc.sync.dma_start(
                                out=h_locB[i + 1][(b - NBLK // 2) * P:
                                                  (b - NBLK // 2 + 1) * P, :],
                                in_=accs[b][:])
                        if b == NBLK // 2 - 1:
                            nc.gpsimd.collective_compute(
                                "AllGather", OP.bypass, replica_groups=groups,
                                ins=[h_locA[i + 1][:]],
                                outs=[h_all[i + 1][0:N // 2, :]])
                    else:
                        # attention score for this block, overlapped with the
                        # remaining layer-2 gathers
                        ps_tt = pp2.tile([P, HID], BF16, tag="pa0")
                        nc.tensor.transpose(ps_tt[:, 0:P], accs[b][:, 0:P], ident[:])
                        nc.tensor.transpose(ps_tt[:, P:HID], accs[b][:, P:HID], ident[:])
                        hT = wk.tile([P, HID], BF16, tag="tT")
                        nc.scalar.copy(hT[:], ps_tt[:])
                        ps_s1 = pp2.tile([P, P], F32, tag="pz")
                        nc.tensor.matmul(ps_s1[:], lhsT=Wa1_sb[:, 0, :], rhs=hT[:, 0:P],
                                         start=True, stop=False)
                        nc.tensor.matmul(ps_s1[:], lhsT=Wa1_sb[:, 1, :], rhs=hT[:, P:HID],
                                         start=False, stop=False)
                        nc.tensor.matmul(ps_s1[:], lhsT=ba1_sb[:], rhs=ones_bf[:],
                                         start=False, stop=True)
                        s1t = wk.tile([P, P], BF16, tag="s1t")
                        nc.scalar.activation(out=s1t[:], in_=ps_s1[:], func=AF.Tanh)
                        ps_sc = pp1.tile([P, 1], F32, tag="pe")
                        nc.tensor.matmul(ps_sc[:], lhsT=s1t[:], rhs=Wa2_sb[:],
                                         start=True, stop=False)
                        nc.tensor.matmul(ps_sc[:], lhsT=ones_bf[:], rhs=ba2_sb[:],
                                         start=False, stop=True)
                        nc.vector.tensor_copy(score_all[:, b:b + 1], ps_sc[:])

            # batched softmax over (tile, type) groups
            ps_scT = pp2.tile([NBLK, P], F32, tag="pz")
            nc.tensor.transpose(ps_scT[:], score_all[:], identf[:])
            scT = big.tile([NBLK, P], F32)
            nc.vector.tensor_copy(scT[:], ps_scT[:])
            mx = big.tile([NBLK, NUM_TYPES], F32)
            nc.vector.tensor_reduce(out=mx[:], in_=scT[:].rearrange("p (k t) -> p t k", t=4),
                                    axis=mybir.AxisListType.X, op=OP.max)
            exr = big.tile([NBLK, P], F32)
            nc.vector.tensor_tensor(out=exr[:].rearrange("p (k t) -> p t k", t=4),
                                    in0=scT[:].rearrange("p (k t) -> p t k", t=4),
                                    in1=mx[:].to_broadcast([NBLK, NUM_TYPES, 32]),
                                    op=OP.subtract)
            nc.scalar.activation(out=exr[:], in_=exr[:], func=AF.Exp)
            den = big.tile([NBLK, NUM_TYPES], F32)
            nc.vector.tensor_reduce(out=den[:], in_=exr[:].rearrange("p (k t) -> p t k", t=4),
                                    axis=mybir.AxisListType.X, op=OP.add)
            nc.vector.reciprocal(den[:], den[:])
            attnT = big.tile([NBLK, P], F32)
            nc.vector.tensor_tensor(out=attnT[:].rearrange("p (k t) -> p t k", t=4),
                                    in0=exr[:].rearrange("p (k t) -> p t k", t=4),
                                    in1=den[:].to_broadcast([NBLK, NUM_TYPES, 32]),
                                    op=OP.mult)
            ps_at = pp2.tile([P, NBLK], F32, tag="pz")
            nc.tensor.transpose(ps_at[:], attnT[:], identf[0:NBLK, 0:NBLK])
            attn = big.tile([P, NBLK], F32)
            nc.vector.tensor_copy(attn[:], ps_at[:])

            pooled = [big.tile([P, HID], BF16, name=f"pooled{k}") for k in range(2)]
            for b in range(NBLK):
                wm = smp.tile([P, NUM_TYPES], BF16, tag="wm")
                nc.vector.tensor_scalar(out=wm[:], in0=typemask_sb[:],
                                        scalar1=attn[:, b:b + 1], scalar2=None,
                                        op0=OP.mult)
                ps_p = pp1.tile([NUM_TYPES, HID], F32, tag="pe")
                nc.tensor.matmul(ps_p[:], lhsT=wm[:], rhs=accs[b][:], start=True, stop=True)
                pstg = smp.tile([NUM_TYPES, HID], BF16, tag="pstg")
                nc.vector.tensor_copy(pstg[:], ps_p[:])
                half, bl = b // 32, b % 32
                nc.sync.dma_start(out=pooled[half][bl * 4:(bl + 1) * 4, :], in_=pstg[:])

            for half in range(2):
                pa = pooled[half]
                glm = big.tile([P, HID], BF16)
                nc.vector.tensor_tensor(out=glm[:], in0=pa[:], in1=wg_sb[:], op=OP.mult)
                gl = big.tile([P, 1], F32)
                nc.vector.tensor_reduce(out=gl[:], in_=glm[:],
                                        axis=mybir.AxisListType.X, op=OP.add)
                nc.vector.tensor_scalar(out=gl[:], in0=gl[:], scalar1=bg_sb[:],
                                        scalar2=None, op0=OP.add)
                ps_gr = pp1.tile([1, P], F32, tag="pe")
                nc.tensor.transpose(ps_gr[:], gl[:], identf[:])
                glT = big.tile([1, P], F32)
                nc.vector.tensor_copy(glT[:], ps_gr[:])
                mxg = big.tile([1, 32], F32)
                nc.vector.tensor_reduce(out=mxg[:], in_=glT[:].rearrange("p (b t) -> p b t", t=4),
                                        axis=mybir.AxisListType.X, op=OP.max)
                exg = big.tile([1, P], F32)
                nc.vector.tensor_tensor(out=exg[:].rearrange("p (b t) -> p b t", t=4),
                                        in0=glT[:].rearrange("p (b t) -> p b t", t=4),
                                        in1=mxg[:].to_broadcast([1, 32, 4]),
                                        op=OP.subtract)
                nc.scalar.activation(out=exg[:], in_=exg[:], func=AF.Exp)
                deng = big.tile([1, 32], F32)
                nc.vector.tensor_reduce(out=deng[:], in_=exg[:].rearrange("p (b t) -> p b t", t=4),
                                        axis=mybir.AxisListType.X, op=OP.add)
                nc.vector.reciprocal(deng[:], deng[:])
                gates_r = big.tile([1, P], BF16)
                nc.vector.tensor_tensor(out=gates_r[:].rearrange("p (b t) -> p b t", t=4),
                                        in0=exg[:].rearrange("p (b t) -> p b t", t=4),
                                        in1=deng[:].to_broadcast([1, 32, 4]),
                                        op=OP.mult)
                ps_gc = pp1.tile([P, 1], F32, tag="pe")
                nc.tensor.matmul(ps_gc[:], lhsT=gates_r[:], rhs=ones11_bf[:],
                                 start=True, stop=True)
                gc = big.tile([P, 1], F32)
                nc.vector.tensor_copy(gc[:], ps_gc[:])
                psc_sb = big.tile([P, HID], BF16)
                nc.vector.tensor_scalar(out=psc_sb[:], in0=pa[:], scalar1=gc[:],
                                        scalar2=None, op0=OP.mult)
                ps_o = pp1.tile([32, HID], F32, tag="pe")
                nc.tensor.matmul(ps_o[:], lhsT=summat_sb[:], rhs=psc_sb[:],
                                 start=True, stop=True)
                out_sb = big.tile([32, HID], F32)
                nc.vector.tensor_copy(out_sb[:], ps_o[:])
                nc.sync.dma_start(out=out_dram[half * 32:(half + 1) * 32, :], in_=out_sb[:])

    nc.compile()
    return nc


def kernel(**inputs):
    per_core, weights, T_blk, T_tot, tile_start = _prep(**inputs)
    key = (T_tot, tuple(int(t) for t in T_blk))
    if key not in _PROG_CACHE:
        _PROG_CACHE[key] = _build(T_blk, T_tot, tile_start)
    nc = _PROG_CACHE[key]
    in_maps = []
    for c in range(NCORES):
        m = dict(weights)
        pc = per_core[c]
        m.update(xT_aug=pc["xT_aug"], gsrc=pc["gsrc"], onehot=pc["onehot"],
                 ea_aug=pc["ea_aug"], time_rows=pc["time_rows"])
        in_maps.append(m)
    res = run_bass_kernel_spmd(nc, in_maps, list(range(NCORES)), trace=False)
    out = np.concatenate(
        [res.results[c]["out"].reshape(NBLK // T, T, HID) for c in range(NCORES)],
        axis=0)
    return np.ascontiguousarray(out.astype(np.float32))


# revision 27
# speedup vs baseline: 1.0016x; 1.0016x over previous
"""Trainium2 Bass kernel for nn_GCNBackbone (3-layer GNN + attention pooling).

Self-contained: accepts FULL inputs, shards across 8 NeuronCores by dst-node
blocks (whole graphs per core), runs a Bass/Tile SPMD program, gathers the
full output.

Key algebra: msg = concat(h[src], ea) @ W + b summed over incoming edges
  == (sum h[src]) @ W_h + (sum ea) @ W_e + deg * b
so the per-edge matmul collapses to per-node matmuls; the irregular part is
a gather of h rows plus a segmented sum done as one-hot matmuls on PE.
"""
import numpy as np
import ml_dtypes

import concourse.bacc as bacc
import concourse.bass as bass
import concourse.tile as tile
from concourse import mybir
from concourse.bass_utils import run_bass_kernel_spmd
from concourse.masks import make_identity

F32 = mybir.dt.float32
BF16 = mybir.dt.bfloat16
I32 = mybir.dt.int32
AF = mybir.ActivationFunctionType
OP = mybir.AluOpType

B, T, K = 32, 16, 128
NUM_TYPES = 4
N = B * T * K
E_GLOBAL = 262144
IN_DIM, EDGE_DIM, TIME_DIM = 64, 32, 32
ENC_DIM, HID = 224, 256
NUM_LAYERS = 3
NCORES = 8
NPC = N // NCORES
NBLK = NPC // 128
P = 128
EPS = 1e-5

_PROG_CACHE = {}


def _bcast_rows(ap, p=P):
    """DRAM row AP -> partition-broadcast AP [[0,p], free...]."""
    return bass.AP(tensor=ap.tensor, offset=ap.offset, ap=[[0, p]] + list(ap.ap[1:]))


def _prep(x, edge_attr, time_emb, params, edge_index, batch_idx, temporal, node_type):
    x = np.ascontiguousarray(np.asarray(x, np.float32))
    edge_attr = np.ascontiguousarray(np.asarray(edge_attr, np.float32))
    time_emb = np.ascontiguousarray(np.asarray(time_emb, np.float32))
    ei = np.asarray(edge_index, np.int64)
    src, dst = ei[0], ei[1]

    node = np.arange(N, dtype=np.int64)
    assert np.array_equal(np.asarray(batch_idx, np.int64), node // (T * K))
    assert np.array_equal(np.asarray(temporal, np.int64), (node // K) % T)
    assert np.array_equal(np.asarray(node_type, np.int64), node % NUM_TYPES)

    core_of = dst // NPC
    blk_of = (dst % NPC) // P
    cnt = np.zeros((NCORES, NBLK), np.int64)
    for c in range(NCORES):
        cnt[c] = np.bincount(blk_of[core_of == c], minlength=NBLK)
    T_blk = np.maximum(1, -(-cnt // P)).max(axis=0)       # per-block max over cores
    T_tot = int(T_blk.sum())
    tile_start = np.zeros(NBLK + 1, np.int64)
    tile_start[1:] = np.cumsum(T_blk)

    per_core = []
    for c in range(NCORES):
        m = np.nonzero(core_of == c)[0]
        order = m[np.argsort(blk_of[m], kind="stable")]
        gsrc = np.zeros((P, T_tot), np.int32)
        onehot = np.zeros((P, T_tot, P), np.float32)
        ea_aug = np.zeros((T_tot * P, EDGE_DIM + 1), np.float32)
        pos = 0
        for b in range(NBLK):
            nb = int(cnt[c, b])
            eb = order[pos:pos + nb]
            pos += nb
            if nb:
                dloc = ((dst[eb] % NPC) % P).astype(np.int64)
                i = np.arange(nb)
                tt = tile_start[b] + i // P
                pp = i % P
                sc = src[eb] // NPC
                sr = src[eb] % NPC
                half = NPC // 2
                gsrc[pp, tt] = np.where(
                    sr < half, sc * half + sr,
                    N // 2 + sc * half + (sr - half)).astype(np.int32)
                onehot[pp, tt, dloc] = 1.0
                ea_aug[tt * P + pp, :EDGE_DIM] = edge_attr[eb]
                ea_aug[tt * P + pp, EDGE_DIM] = 1.0
        assert pos == len(order)
        xc = x[c * NPC:(c + 1) * NPC]
        xT_aug = np.ones((IN_DIM + 1, NPC), np.float32)
        xT_aug[:IN_DIM] = xc.T
        g_ids = (c * NPC + np.arange(NBLK) * P) // (T * K)
        time_rows = time_emb[g_ids]                        # [64, 32]
        per_core.append(dict(
            gsrc=gsrc,
            onehot=np.ascontiguousarray(onehot.reshape(P, T_tot * P)).astype(ml_dtypes.bfloat16),
            ea_aug=ea_aug,
            xT_aug=xT_aug,
            time_rows=np.ascontiguousarray(time_rows),
        ))

    p_ = {k: np.asarray(v, np.float32) for k, v in params.items()}
    Ws = [p_["W_c0"], p_["W_c1"], p_["W_c2"]]
    bs = [p_["b_c0"], p_["b_c1"], p_["b_c2"]]
    tm = np.zeros((P, NUM_TYPES), np.float32)
    tm[np.arange(P), np.arange(P) % NUM_TYPES] = 1.0
    sm = np.zeros((P, 32), np.float32)
    sm[np.arange(P), np.arange(P) // 4] = 1.0
    weights = dict(
        W1_aug=np.ascontiguousarray(np.vstack([p_["W_enc1"], p_["b_enc1"][None, :]])),
        W_enc2=p_["W_enc2"],
        genc_col=np.ascontiguousarray(p_["g_enc"][:, None]),
        benc_col=np.ascontiguousarray(p_["be_enc"][:, None]),
        b_enc2_row=np.ascontiguousarray(p_["b_enc2"][None, :]),
        W_h_all=np.ascontiguousarray(np.stack([Ws[i][:HID] for i in range(3)])),
        W_e_all=np.ascontiguousarray(
            np.stack([np.vstack([Ws[i][HID:], bs[i][None, :]]) for i in range(3)])),
        ln_g_all=np.ascontiguousarray(p_["ln_g"]),
        ln_b_all=np.ascontiguousarray(p_["ln_b"]),
        W_a1=p_["W_a1"],
        b_a1_row=np.ascontiguousarray(p_["b_a1"][None, :]),
        W_a2_col=np.ascontiguousarray(p_["W_a2"]),
        b_a2=np.ascontiguousarray(p_["b_a2"][None, :]),      # [1,1]
        W_g_row=np.ascontiguousarray(p_["W_g"].T),           # [1,256]
        b_g=np.ascontiguousarray(p_["b_g"][None, :]),        # [1,1]
        typemask=tm.astype(ml_dtypes.bfloat16),
        summat=sm.astype(ml_dtypes.bfloat16),
    )
    return per_core, weights, T_blk, T_tot, tile_start


def _build(T_blk, T_tot, tile_start):
    nc = bacc.Bacc(None, target_bir_lowering=False, num_devices=NCORES,
                   dynamic_dma_scratch_size=32768)

    dp = nc.declare_dram_parameter
    xT_aug = dp("xT_aug", [IN_DIM + 1, NPC], F32, isOutput=False)
    gsrc = dp("gsrc", [P, T_tot], I32, isOutput=False)
    onehot = dp("onehot", [P, T_tot * P], BF16, isOutput=False)
    ea_aug = dp("ea_aug", [T_tot * P, EDGE_DIM + 1], F32, isOutput=False)
    time_rows = dp("time_rows", [NBLK, TIME_DIM], F32, isOutput=False)
    W1_aug = dp("W1_aug", [IN_DIM + 1, HID], F32, isOutput=False)
    W_enc2 = dp("W_enc2", [HID, ENC_DIM], F32, isOutput=False)
    genc_col = dp("genc_col", [HID, 1], F32, isOutput=False)
    benc_col = dp("benc_col", [HID, 1], F32, isOutput=False)
    b_enc2_row = dp("b_enc2_row", [1, ENC_DIM], F32, isOutput=False)
    W_h_all = dp("W_h_all", [3, HID, HID], F32, isOutput=False)
    W_e_all = dp("W_e_all", [3, EDGE_DIM + 1, HID], F32, isOutput=False)
    ln_g_all = dp("ln_g_all", [3, HID], F32, isOutput=False)
    ln_b_all = dp("ln_b_all", [3, HID], F32, isOutput=False)
    W_a1 = dp("W_a1", [HID, P], F32, isOutput=False)
    b_a1_row = dp("b_a1_row", [1, P], F32, isOutput=False)
    W_a2_col = dp("W_a2_col", [P, 1], F32, isOutput=False)
    b_a2_in = dp("b_a2", [1, 1], F32, isOutput=False)
    W_g_row = dp("W_g_row", [1, HID], F32, isOutput=False)
    b_g_in = dp("b_g", [1, 1], F32, isOutput=False)
    typemask_in = dp("typemask", [P, NUM_TYPES], BF16, isOutput=False)
    summat_in = dp("summat", [P, 32], BF16, isOutput=False)
    out_dram = dp("out", [NBLK, HID], F32, isOutput=True)

    # bf16 shadows in DRAM (SWDGE cast once, HWDGE loads after)
    xT_bf = nc.dram_tensor("xT_bf", [IN_DIM + 1, NPC], BF16)
    ea_bf = nc.dram_tensor("ea_bf", [T_tot * P, EDGE_DIM + 1], BF16)
    W1_bf = nc.dram_tensor("W1_bf", [IN_DIM + 1, HID], BF16)
    Wh_bf = nc.dram_tensor("Wh_bf", [3, HID, HID], BF16)
    We_bf = nc.dram_tensor("We_bf", [3, EDGE_DIM + 1, HID], BF16)
    Wa1_bf = nc.dram_tensor("Wa1_bf", [HID, P], BF16)
    ba1_bf_d = nc.dram_tensor("ba1_bf", [1, P], BF16)
    Wa2_bf_d = nc.dram_tensor("Wa2_bf", [P, 1], BF16)
    ba2_bf_d = nc.dram_tensor("ba2_bf", [1, 1], BF16)
    tmr_bf_d = nc.dram_tensor("tmr_bf", [1, NBLK * TIME_DIM], BF16)

    h_locA = [nc.dram_tensor(f"h_locA{i}", [NPC // 2, HID], BF16) for i in range(3)]
    h_locB = [nc.dram_tensor(f"h_locB{i}", [NPC // 2, HID], BF16) for i in range(3)]
    h_all = [nc.dram_tensor(f"h_all{i}", [N, HID], BF16, addr_space="Shared")
             for i in range(3)]
    groups = [list(range(NCORES))]

    with tile.TileContext(nc) as tc:
        with (
            tc.tile_pool(name="big", bufs=1) as big,
            tc.tile_pool(name="acc", bufs=1) as accp,
            tc.tile_pool(name="gath", bufs=16) as gp,
            tc.tile_pool(name="work", bufs=4) as wk,
            tc.tile_pool(name="sm", bufs=8) as smp,
            tc.tile_pool(name="pp2", bufs=2, space="PSUM") as pp2,
            tc.tile_pool(name="pp1", bufs=1, space="PSUM") as pp1,
        ):
            # ---------- one-time casts (SWDGE) ----------
            nc.gpsimd.dma_start(out=xT_bf[:], in_=xT_aug[:])
            nc.gpsimd.dma_start(out=ea_bf[:], in_=ea_aug[:])
            nc.gpsimd.dma_start(out=W1_bf[:], in_=W1_aug[:])
            nc.gpsimd.dma_start(out=Wh_bf[:], in_=W_h_all[:])
            nc.gpsimd.dma_start(out=We_bf[:], in_=W_e_all[:])
            nc.gpsimd.dma_start(out=Wa1_bf[:], in_=W_a1[:])
            nc.gpsimd.dma_start(out=ba1_bf_d[:], in_=b_a1_row[:])
            nc.gpsimd.dma_start(out=Wa2_bf_d[:], in_=W_a2_col[:])
            nc.gpsimd.dma_start(out=ba2_bf_d[:], in_=b_a2_in[:])
            nc.gpsimd.dma_start(out=tmr_bf_d[:],
                                in_=time_rows[:].rearrange("b t -> (b t)")[None, :])

            # ---------- resident SBUF state ----------
            onehot_sb = big.tile([P, T_tot * P], BF16)
            nc.sync.dma_start(out=onehot_sb[:], in_=onehot[:])
            gsrc_sb = big.tile([P, T_tot], I32)
            nc.sync.dma_start(out=gsrc_sb[:], in_=gsrc[:])
            aggTe_sb = big.tile([EDGE_DIM + 1, NPC], BF16)
            W1_sb = big.tile([IN_DIM + 1, HID], BF16)
            nc.sync.dma_start(out=W1_sb[:], in_=W1_bf[:])
            Wh_sb = big.tile([P, 3, 2, HID], BF16)
            nc.sync.dma_start(out=Wh_sb[:],
                              in_=Wh_bf[:].rearrange("l (c p) n -> p l c n", p=P))
            We_sb = big.tile([EDGE_DIM + 1, 3, HID], BF16)
            nc.sync.dma_start(out=We_sb[:],
                              in_=We_bf[:].rearrange("l e n -> e l n"))
            Wa1_sb = big.tile([P, 2, P], BF16)
            nc.sync.dma_start(out=Wa1_sb[:],
                              in_=Wa1_bf[:].rearrange("(c p) a -> p c a", p=P))
            ba1_sb = big.tile([1, P], BF16)
            nc.sync.dma_start(out=ba1_sb[:], in_=ba1_bf_d[:])
            Wa2_sb = big.tile([P, 1], BF16)
            nc.sync.dma_start(out=Wa2_sb[:], in_=Wa2_bf_d[:])
            ba2_sb = big.tile([1, 1], BF16)
            nc.sync.dma_start(out=ba2_sb[:], in_=ba2_bf_d[:])
            time_sb = big.tile([1, NBLK * TIME_DIM], BF16)
            nc.sync.dma_start(out=time_sb[:], in_=tmr_bf_d[:])
            typemask_sb = big.tile([P, NUM_TYPES], BF16)
            nc.sync.dma_start(out=typemask_sb[:], in_=typemask_in[:])
            summat_sb = big.tile([P, 32], BF16)
            nc.sync.dma_start(out=summat_sb[:], in_=summat_in[:])
            # broadcast loads (SWDGE cast f32->bf16, partition step 0)
            lng_sb = big.tile([P, 3, HID], BF16)
            lnb_sb = big.tile([P, 3, HID], BF16)
            for i in range(3):
                nc.gpsimd.dma_start(out=lng_sb[:, i, :], in_=_bcast_rows(ln_g_all[i:i + 1, :]))
                nc.gpsimd.dma_start(out=lnb_sb[:, i, :], in_=_bcast_rows(ln_b_all[i:i + 1, :]))
            wg_sb = big.tile([P, HID], BF16)
            nc.gpsimd.dma_start(out=wg_sb[:], in_=_bcast_rows(W_g_row[:]))
            bg_sb = big.tile([P, 1], F32)
            nc.gpsimd.dma_start(out=bg_sb[:], in_=_bcast_rows(b_g_in[:]))

            eps_sb = big.tile([P, 1], F32)
            nc.vector.memset(eps_sb[:], EPS)
            ones_bf = big.tile([1, P], BF16)
            nc.vector.memset(ones_bf[:], 1.0)
            ones11_bf = big.tile([1, 1], BF16)
            nc.vector.memset(ones11_bf[:], 1.0)
            ones11_f = big.tile([1, 1], F32)
            nc.vector.memset(ones11_f[:], 1.0)
            ident = big.tile([P, P], BF16)
            make_identity(nc, ident[:])
            identf = big.tile([P, P], F32)
            make_identity(nc, identf[:])

            # ---------- encoder weight folds (device, f32) ----------
            W2_sb = big.tile([P, 2, ENC_DIM], F32)
            nc.sync.dma_start(out=W2_sb[:], in_=W_enc2[:].rearrange("(c p) n -> p c n", p=P))
            b2r_sb = big.tile([1, ENC_DIM], F32)
            nc.sync.dma_start(out=b2r_sb[:], in_=b_enc2_row[:])
            W2p_sb = big.tile([P, 2, ENC_DIM], BF16)
            b2p_sb = big.tile([1, ENC_DIM], BF16)
            ps_b2 = pp1.tile([1, ENC_DIM], F32, tag="psc")
            for k2 in range(2):
                gch = smp.tile([P, 1], F32, tag="gch")
                nc.sync.dma_start(out=gch[:], in_=genc_col[k2 * P:(k2 + 1) * P, :])
                nc.vector.tensor_scalar(out=W2p_sb[:, k2, :], in0=W2_sb[:, k2, :],
                                        scalar1=gch[:], scalar2=None, op0=OP.mult)
                bch = smp.tile([P, 1], F32, tag="bch")
                nc.sync.dma_start(out=bch[:], in_=benc_col[k2 * P:(k2 + 1) * P, :])
                nc.tensor.matmul(ps_b2[:], lhsT=bch[:], rhs=W2_sb[:, k2, :],
                                 start=(k2 == 0), stop=False)
            nc.tensor.matmul(ps_b2[:], lhsT=ones11_f[:], rhs=b2r_sb[:],
                             start=False, stop=True)
            nc.scalar.copy(b2p_sb[:], ps_b2[:])

            accs = [accp.tile([P, HID], BF16, tag=f"a{b}", name=f"acc{b}")
                    for b in range(NBLK)]

            # ---------- encoder ----------
            for b in range(NBLK):
                xt = wk.tile([IN_DIM + 1, P], BF16, tag="xt")
                nc.sync.dma_start(out=xt[:], in_=xT_bf[:, b * P:(b + 1) * P])
                ps_h1 = pp2.tile([P, HID], F32, tag="pa1")
                nc.tensor.matmul(ps_h1[:], lhsT=xt[:], rhs=W1_sb[:], start=True, stop=True)
                h1 = wk.tile([P, HID], F32, tag="h1")
                nc.scalar.activation(out=h1[:], in_=ps_h1[:], func=AF.Relu)
                stats = smp.tile([P, 6], F32, tag="st")
                nc.vector.bn_stats(out=stats[:], in_=h1[:])
                mv = smp.tile([P, 2], F32, tag="mv")
                nc.vector.bn_aggr(out=mv[:], in_=stats[:])
                rstd = smp.tile([P, 1], F32, tag="rstd")
                nc.scalar.activation(out=rstd[:], in_=mv[:, 1:2], func=AF.Sqrt,
                                     bias=eps_sb[:], scale=1.0)
                nc.vector.reciprocal(rstd[:], rstd[:])
                nm = smp.tile([P, 1], F32, tag="nm")
                nc.vector.tensor_scalar(out=nm[:], in0=mv[:, 0:1], scalar1=rstd[:],
                                        scalar2=-1.0, op0=OP.mult, op1=OP.mult)
                h1n0 = wk.tile([P, HID], BF16, tag="h1n0")
                nc.vector.tensor_scalar(out=h1n0[:], in0=h1[:], scalar1=rstd[:],
                                        scalar2=nm[:], op0=OP.mult, op1=OP.add)
                ps_tt = pp2.tile([P, HID], BF16, tag="pa0")
                nc.tensor.transpose(ps_tt[:, 0:P], h1n0[:, 0:P], ident[:])
                nc.tensor.transpose(ps_tt[:, P:HID], h1n0[:, P:HID], ident[:])
                tT = wk.tile([P, HID], BF16, tag="tT")
                nc.scalar.copy(tT[:], ps_tt[:])
                ps_he = pp2.tile([P, HID], F32, tag="pz")
                nc.tensor.matmul(ps_he[:, :ENC_DIM], lhsT=tT[:, 0:P], rhs=W2p_sb[:, 0, :],
                                 start=True, stop=False)
                nc.tensor.matmul(ps_he[:, :ENC_DIM], lhsT=tT[:, P:HID], rhs=W2p_sb[:, 1, :],
                                 start=False, stop=False)
                nc.tensor.matmul(ps_he[:, :ENC_DIM], lhsT=ones_bf[:], rhs=b2p_sb[:],
                                 start=False, stop=True)
                nc.tensor.matmul(ps_he[:, ENC_DIM:], lhsT=ones_bf[:],
                                 rhs=time_sb[:, b * TIME_DIM:(b + 1) * TIME_DIM],
                                 start=True, stop=True)
                henc = wk.tile([P, HID], BF16, tag="henc")
                nc.vector.tensor_copy(henc[:], ps_he[:])
                if b < NBLK // 2:
                    nc.sync.dma_start(out=h_locA[0][b * P:(b + 1) * P, :], in_=henc[:])
                else:
                    nc.sync.dma_start(
                        out=h_locB[0][(b - NBLK // 2) * P:(b - NBLK // 2 + 1) * P, :],
                        in_=henc[:])
                if b == NBLK // 2 - 1:
                    nc.gpsimd.collective_compute(
                        "AllGather", OP.bypass, replica_groups=groups,
                        ins=[h_locA[0][:]], outs=[h_all[0][0:N // 2, :]])

            # ---------- layers ----------
            score_all = big.tile([P, NBLK], F32)
            for i in range(NUM_LAYERS):
                nc.gpsimd.collective_compute(
                    "AllGather", OP.bypass, replica_groups=groups,
                    ins=[h_locB[i][:]], outs=[h_all[i][N // 2:N, :]])
                if i == 0:
                    # aggTe pass (layer-independent) — fills the AG idle window
                    for b in range(NBLK):
                        ps_e = pp1.tile([EDGE_DIM + 1, P], F32, tag="pe")
                        t0, t1 = int(tile_start[b]), int(tile_start[b + 1])
                        for t in range(t0, t1):
                            ea_t = wk.tile([P, EDGE_DIM + 1], BF16, tag="ea")
                            nc.sync.dma_start(out=ea_t[:], in_=ea_bf[t * P:(t + 1) * P, :])
                            nc.tensor.matmul(ps_e[:], lhsT=ea_t[:],
                                             rhs=onehot_sb[:, t * P:(t + 1) * P],
                                             start=(t == t0), stop=(t == t1 - 1))
                        nc.vector.tensor_copy(aggTe_sb[:, b * P:(b + 1) * P], ps_e[:])

                for b in range(NBLK):
                    t0, t1 = int(tile_start[b]), int(tile_start[b + 1])
                    ps_a0 = pp2.tile([P, P], F32, tag="pa0")
                    ps_a1 = pp2.tile([P, P], F32, tag="pa1")
                    for t in range(t0, t1):
                        g = gp.tile([P, HID], BF16, tag="g")
                        nc.gpsimd.indirect_dma_start(
                            out=g[:], out_offset=None, in_=h_all[i][:],
                            in_offset=bass.IndirectOffsetOnAxis(
                                ap=gsrc_sb[:, t:t + 1], axis=0))
                        oh = onehot_sb[:, t * P:(t + 1) * P]
                        nc.tensor.matmul(ps_a0[:], lhsT=g[:, 0:P], rhs=oh,
                                         start=(t == t0), stop=(t == t1 - 1))
                        nc.tensor.matmul(ps_a1[:], lhsT=g[:, P:HID], rhs=oh,
                                         start=(t == t0), stop=(t == t1 - 1))
                    a0 = wk.tile([P, HID], BF16, tag="a0")
                    nc.vector.tensor_copy(a0[:, 0:P], ps_a0[:])
                    nc.vector.tensor_copy(a0[:, P:HID], ps_a1[:])
                    ps_z = pp2.tile([P, HID], F32, tag="pz")
                    nc.tensor.matmul(ps_z[:], lhsT=a0[:, 0:P], rhs=Wh_sb[:, i, 0, :],
                                     start=True, stop=False)
                    nc.tensor.matmul(ps_z[:], lhsT=a0[:, P:HID], rhs=Wh_sb[:, i, 1, :],
                                     start=False, stop=False)
                    nc.tensor.matmul(ps_z[:], lhsT=aggTe_sb[:, b * P:(b + 1) * P],
                                     rhs=We_sb[:, i, :], start=False, stop=True)
                    stats = smp.tile([P, 6], F32, tag="st")
                    nc.vector.bn_stats(out=stats[:], in_=ps_z[:])
                    mv = smp.tile([P, 2], F32, tag="mv")
                    nc.vector.bn_aggr(out=mv[:], in_=stats[:])
                    rstd = smp.tile([P, 1], F32, tag="rstd")
                    nc.scalar.activation(out=rstd[:], in_=mv[:, 1:2], func=AF.Sqrt,
                                         bias=eps_sb[:], scale=1.0)
                    nc.vector.reciprocal(rstd[:], rstd[:])
                    nm = smp.tile([P, 1], F32, tag="nm")
                    nc.vector.tensor_scalar(out=nm[:], in0=mv[:, 0:1], scalar1=rstd[:],
                                            scalar2=-1.0, op0=OP.mult, op1=OP.mult)
                    tnorm = wk.tile([P, HID], BF16, tag="tn")
                    nc.scalar.activation(out=tnorm[:], in_=ps_z[:], func=AF.Identity,
                                         bias=nm[:], scale=rstd[:])
                    u = wk.tile([P, HID], BF16, tag="u")
                    nc.vector.tensor_tensor(out=u[:], in0=tnorm[:], in1=lng_sb[:, i, :],
                                            op=OP.mult)
                    v = wk.tile([P, HID], BF16, tag="v")
                    nc.vector.tensor_tensor(out=v[:], in0=u[:], in1=lnb_sb[:, i, :],
                                            op=OP.add)
                    if i == 0:
                        nc.scalar.activation(out=accs[b][:], in_=v[:], func=AF.Silu)
                    else:
                        hn = wk.tile([P, HID], BF16, tag="hn")
                        nc.scalar.activation(out=hn[:], in_=v[:], func=AF.Silu)
                        nc.vector.tensor_tensor(out=accs[b][:], in0=accs[b][:],
                                                in1=hn[:], op=OP.add)
                    if i < 2:
                        if b < NBLK // 2:
                            nc.sync.dma_start(out=h_locA[i + 1][b * P:(b + 1) * P, :],
                                              in_=accs[b][:])
                        else:
                            nc.sync.dma_start(
                                out=h_locB[i + 1][(b - NBLK // 2) * P:
                                                  (b - NBLK // 2 + 1) * P, :],
                                in_=accs[b][:])
                        if b == NBLK // 2 - 1:
                            nc.gpsimd.collective_compute(
                                "AllGather", OP.bypass, replica_groups=groups,
                                ins=[h_locA[i + 1][:]],
                                outs=[h_all[i + 1][0:N // 2, :]])
                    else:
                        # attention score for this block, overlapped with the
                        # remaining layer-2 gathers
                        ps_tt = pp2.tile([P, HID], BF16, tag="pa0")
                        nc.tensor.transpose(ps_tt[:, 0:P], accs[b][:, 0:P], ident[:])
                        nc.tensor.transpose(ps_tt[:, P:HID], accs[b][:, P:HID], ident[:])
                        hT = wk.tile([P, HID], BF16, tag="tT")
                        nc.scalar.copy(hT[:], ps_tt[:])
                        ps_s1 = pp2.tile([P, P], F32, tag="pz")
                        nc.tensor.matmul(ps_s1[:], lhsT=Wa1_sb[:, 0, :], rhs=hT[:, 0:P],
                                         start=True, stop=False)
                        nc.tensor.matmul(ps_s1[:], lhsT=Wa1_sb[:, 1, :], rhs=hT[:, P:HID],
                                         start=False, stop=False)
                        nc.tensor.matmul(ps_s1[:], lhsT=ba1_sb[:], rhs=ones_bf[:],
                                         start=False, stop=True)
                        s1t = wk.tile([P, P], BF16, tag="s1t")
                        nc.scalar.activation(out=s1t[:], in_=ps_s1[:], func=AF.Tanh)
                        ps_sc = pp1.tile([P, 1], F32, tag="psc")
                        nc.tensor.matmul(ps_sc[:], lhsT=s1t[:], rhs=Wa2_sb[:],
                                         start=True, stop=False)
                        nc.tensor.matmul(ps_sc[:], lhsT=ones_bf[:], rhs=ba2_sb[:],
                                         start=False, stop=True)
                        nc.vector.tensor_copy(score_all[:, b:b + 1], ps_sc[:])

            # batched softmax over (tile, type) groups
            ps_scT = pp2.tile([NBLK, P], F32, tag="pz")
            nc.tensor.transpose(ps_scT[:], score_all[:], identf[:])
            scT = big.tile([NBLK, P], F32)
            nc.vector.tensor_copy(scT[:], ps_scT[:])
            mx = big.tile([NBLK, NUM_TYPES], F32)
            nc.vector.tensor_reduce(out=mx[:], in_=scT[:].rearrange("p (k t) -> p t k", t=4),
                                    axis=mybir.AxisListType.X, op=OP.max)
            exr = big.tile([NBLK, P], F32)
            nc.vector.tensor_tensor(out=exr[:].rearrange("p (k t) -> p t k", t=4),
                                    in0=scT[:].rearrange("p (k t) -> p t k", t=4),
                                    in1=mx[:].to_broadcast([NBLK, NUM_TYPES, 32]),
                                    op=OP.subtract)
            nc.scalar.activation(out=exr[:], in_=exr[:], func=AF.Exp)
            den = big.tile([NBLK, NUM_TYPES], F32)
            nc.vector.tensor_reduce(out=den[:], in_=exr[:].rearrange("p (k t) -> p t k", t=4),
                                    axis=mybir.AxisListType.X, op=OP.add)
            nc.vector.reciprocal(den[:], den[:])
            attnT = big.tile([NBLK, P], F32)
            nc.vector.tensor_tensor(out=attnT[:].rearrange("p (k t) -> p t k", t=4),
                                    in0=exr[:].rearrange("p (k t) -> p t k", t=4),
                                    in1=den[:].to_broadcast([NBLK, NUM_TYPES, 32]),
                                    op=OP.mult)
            ps_at = pp2.tile([P, NBLK], F32, tag="pz")
            nc.tensor.transpose(ps_at[:], attnT[:], identf[0:NBLK, 0:NBLK])
            attn = big.tile([P, NBLK], F32)
            nc.vector.tensor_copy(attn[:], ps_at[:])

            pooled = [big.tile([P, HID], BF16, name=f"pooled{k}") for k in range(2)]
            for b in range(NBLK):
                wm = smp.tile([P, NUM_TYPES], BF16, tag="wm")
                nc.vector.tensor_scalar(out=wm[:], in0=typemask_sb[:],
                                        scalar1=attn[:, b:b + 1], scalar2=None,
                                        op0=OP.mult)
                ps_p = pp1.tile([NUM_TYPES, HID], F32, tag="pe")
                nc.tensor.matmul(ps_p[:], lhsT=wm[:], rhs=accs[b][:], start=True, stop=True)
                pstg = smp.tile([NUM_TYPES, HID], BF16, tag="pstg")
                nc.vector.tensor_copy(pstg[:], ps_p[:])
                half, bl = b // 32, b % 32
                nc.sync.dma_start(out=pooled[half][bl * 4:(bl + 1) * 4, :], in_=pstg[:])

            for half in range(2):
                pa = pooled[half]
                glm = big.tile([P, HID], BF16)
                nc.vector.tensor_tensor(out=glm[:], in0=pa[:], in1=wg_sb[:], op=OP.mult)
                gl = big.tile([P, 1], F32)
                nc.vector.tensor_reduce(out=gl[:], in_=glm[:],
                                        axis=mybir.AxisListType.X, op=OP.add)
                nc.vector.tensor_scalar(out=gl[:], in0=gl[:], scalar1=bg_sb[:],
                                        scalar2=None, op0=OP.add)
                ps_gr = pp1.tile([1, P], F32, tag="psc")
                nc.tensor.transpose(ps_gr[:], gl[:], identf[:])
                glT = big.tile([1, P], F32)
                nc.vector.tensor_copy(glT[:], ps_gr[:])
                mxg = big.tile([1, 32], F32)
                nc.vector.tensor_reduce(out=mxg[:], in_=glT[:].rearrange("p (b t) -> p b t", t=4),
                                        axis=mybir.AxisListType.X, op=OP.max)
                exg = big.tile([1, P], F32)
                nc.vector.tensor_tensor(out=exg[:].rearrange("p (b t) -> p b t", t=4),
                                        in0=glT[:].rearrange("p (b t) -> p b t", t=4),
                                        in1=mxg[:].to_broadcast([1, 32, 4]),
                                        op=OP.subtract)
                nc.scalar.activation(out=exg[:], in_=exg[:], func=AF.Exp)
                deng = big.tile([1, 32], F32)
                nc.vector.tensor_reduce(out=deng[:], in_=exg[:].rearrange("p (b t) -> p b t", t=4),
                                        axis=mybir.AxisListType.X, op=OP.add)
                nc.vector.reciprocal(deng[:], deng[:])
                gates_r = big.tile([1, P], BF16)
                nc.vector.tensor_tensor(out=gates_r[:].rearrange("p (b t) -> p b t", t=4),
                                        in0=exg[:].rearrange("p (b t) -> p b t", t=4),
                                        in1=deng[:].to_broadcast([1, 32, 4]),
                                        op=OP.mult)
                ps_gc = pp1.tile([P, 1], F32, tag="psc")
                nc.tensor.matmul(ps_gc[:], lhsT=gates_r[:], rhs=ones11_bf[:],
                                 start=True, stop=True)
                gc = big.tile([P, 1], F32)
                nc.vector.tensor_copy(gc[:], ps_gc[:])
                psc_sb = big.tile([P, HID], BF16)
                nc.vector.tensor_scalar(out=psc_sb[:], in0=pa[:], scalar1=gc[:],
                                        scalar2=None, op0=OP.mult)
                ps_o = pp1.tile([32, HID], F32, tag="pe")
                nc.tensor.matmul(ps_o[:], lhsT=summat_sb[:], rhs=psc_sb[:],
                                 start=True, stop=True)
                out_sb = big.tile([32, HID], F32)
                nc.vector.tensor_copy(out_sb[:], ps_o[:])
                nc.sync.dma_start(out=out_dram[half * 32:(half + 1) * 32, :], in_=out_sb[:])

    nc.compile()
    return nc


def kernel(**inputs):
    per_core, weights, T_blk, T_tot, tile_start = _prep(**inputs)
    key = (T_tot, tuple(int(t) for t in T_blk))
    if key not in _PROG_CACHE:
        _PROG_CACHE[key] = _build(T_blk, T_tot, tile_start)
    nc = _PROG_CACHE[key]
    in_maps = []
    for c in range(NCORES):
        m = dict(weights)
        pc = per_core[c]
        m.update(xT_aug=pc["xT_aug"], gsrc=pc["gsrc"], onehot=pc["onehot"],
                 ea_aug=pc["ea_aug"], time_rows=pc["time_rows"])
        in_maps.append(m)
    res = run_bass_kernel_spmd(nc, in_maps, list(range(NCORES)), trace=False)
    out = np.concatenate(
        [res.results[c]["out"].reshape(NBLK // T, T, HID) for c in range(NCORES)],
        axis=0)
    return np.ascontiguousarray(out.astype(np.float32))


# revision 28
# speedup vs baseline: 1.0237x; 1.0220x over previous
"""Trainium2 Bass kernel for nn_GCNBackbone (3-layer GNN + attention pooling).

Self-contained: accepts FULL inputs, shards across 8 NeuronCores by dst-node
blocks (whole graphs per core), runs a Bass/Tile SPMD program, gathers the
full output.

Key algebra: msg = concat(h[src], ea) @ W + b summed over incoming edges
  == (sum h[src]) @ W_h + (sum ea) @ W_e + deg * b
so the per-edge matmul collapses to per-node matmuls; the irregular part is
a gather of h rows plus a segmented sum done as one-hot matmuls on PE.
"""
import numpy as np
import ml_dtypes

import concourse.bacc as bacc
import concourse.bass as bass
import concourse.tile as tile
from concourse import mybir
from concourse.bass_utils import run_bass_kernel_spmd
from concourse.masks import make_identity

F32 = mybir.dt.float32
BF16 = mybir.dt.bfloat16
I32 = mybir.dt.int32
AF = mybir.ActivationFunctionType
OP = mybir.AluOpType

B, T, K = 32, 16, 128
NUM_TYPES = 4
N = B * T * K
E_GLOBAL = 262144
IN_DIM, EDGE_DIM, TIME_DIM = 64, 32, 32
ENC_DIM, HID = 224, 256
NUM_LAYERS = 3
NCORES = 8
NPC = N // NCORES
NBLK = NPC // 128
P = 128
EPS = 1e-5

_PROG_CACHE = {}


def _bcast_rows(ap, p=P):
    """DRAM row AP -> partition-broadcast AP [[0,p], free...]."""
    return bass.AP(tensor=ap.tensor, offset=ap.offset, ap=[[0, p]] + list(ap.ap[1:]))


def _prep(x, edge_attr, time_emb, params, edge_index, batch_idx, temporal, node_type):
    x = np.ascontiguousarray(np.asarray(x, np.float32))
    edge_attr = np.ascontiguousarray(np.asarray(edge_attr, np.float32))
    time_emb = np.ascontiguousarray(np.asarray(time_emb, np.float32))
    ei = np.asarray(edge_index, np.int64)
    src, dst = ei[0], ei[1]

    node = np.arange(N, dtype=np.int64)
    assert np.array_equal(np.asarray(batch_idx, np.int64), node // (T * K))
    assert np.array_equal(np.asarray(temporal, np.int64), (node // K) % T)
    assert np.array_equal(np.asarray(node_type, np.int64), node % NUM_TYPES)

    core_of = dst // NPC
    blk_of = (dst % NPC) // P
    cnt = np.zeros((NCORES, NBLK), np.int64)
    for c in range(NCORES):
        cnt[c] = np.bincount(blk_of[core_of == c], minlength=NBLK)
    T_blk = np.maximum(1, -(-cnt // P)).max(axis=0)       # per-block max over cores
    T_tot = int(T_blk.sum())
    tile_start = np.zeros(NBLK + 1, np.int64)
    tile_start[1:] = np.cumsum(T_blk)

    per_core = []
    for c in range(NCORES):
        m = np.nonzero(core_of == c)[0]
        order = m[np.argsort(blk_of[m], kind="stable")]
        gsrc = np.zeros((P, T_tot), np.int32)
        onehot = np.zeros((P, T_tot, P), np.float32)
        ea_aug = np.zeros((T_tot * P, EDGE_DIM + 1), np.float32)
        pos = 0
        for b in range(NBLK):
            nb = int(cnt[c, b])
            eb = order[pos:pos + nb]
            pos += nb
            if nb:
                dloc = ((dst[eb] % NPC) % P).astype(np.int64)
                i = np.arange(nb)
                tt = tile_start[b] + i // P
                pp = i % P
                sc = src[eb] // NPC
                sr = src[eb] % NPC
                q = NPC // 4
                gsrc[pp, tt] = ((sr // q) * (N // 4) + sc * q + (sr % q)).astype(np.int32)
                onehot[pp, tt, dloc] = 1.0
                ea_aug[tt * P + pp, :EDGE_DIM] = edge_attr[eb]
                ea_aug[tt * P + pp, EDGE_DIM] = 1.0
        assert pos == len(order)
        xc = x[c * NPC:(c + 1) * NPC]
        xT_aug = np.ones((IN_DIM + 1, NPC), np.float32)
        xT_aug[:IN_DIM] = xc.T
        g_ids = (c * NPC + np.arange(NBLK) * P) // (T * K)
        time_rows = time_emb[g_ids]                        # [64, 32]
        per_core.append(dict(
            gsrc=gsrc,
            onehot=np.ascontiguousarray(onehot.reshape(P, T_tot * P)).astype(ml_dtypes.bfloat16),
            ea_aug=ea_aug,
            xT_aug=xT_aug,
            time_rows=np.ascontiguousarray(time_rows),
        ))

    p_ = {k: np.asarray(v, np.float32) for k, v in params.items()}
    Ws = [p_["W_c0"], p_["W_c1"], p_["W_c2"]]
    bs = [p_["b_c0"], p_["b_c1"], p_["b_c2"]]
    tm = np.zeros((P, NUM_TYPES), np.float32)
    tm[np.arange(P), np.arange(P) % NUM_TYPES] = 1.0
    sm = np.zeros((P, 32), np.float32)
    sm[np.arange(P), np.arange(P) // 4] = 1.0
    weights = dict(
        W1_aug=np.ascontiguousarray(np.vstack([p_["W_enc1"], p_["b_enc1"][None, :]])),
        W_enc2=p_["W_enc2"],
        genc_col=np.ascontiguousarray(p_["g_enc"][:, None]),
        benc_col=np.ascontiguousarray(p_["be_enc"][:, None]),
        b_enc2_row=np.ascontiguousarray(p_["b_enc2"][None, :]),
        W_h_all=np.ascontiguousarray(np.stack([Ws[i][:HID] for i in range(3)])),
        W_e_all=np.ascontiguousarray(
            np.stack([np.vstack([Ws[i][HID:], bs[i][None, :]]) for i in range(3)])),
        ln_g_all=np.ascontiguousarray(p_["ln_g"]),
        ln_b_all=np.ascontiguousarray(p_["ln_b"]),
        W_a1=p_["W_a1"],
        b_a1_row=np.ascontiguousarray(p_["b_a1"][None, :]),
        W_a2_col=np.ascontiguousarray(p_["W_a2"]),
        b_a2=np.ascontiguousarray(p_["b_a2"][None, :]),      # [1,1]
        W_g_row=np.ascontiguousarray(p_["W_g"].T),           # [1,256]
        b_g=np.ascontiguousarray(p_["b_g"][None, :]),        # [1,1]
        typemask=tm.astype(ml_dtypes.bfloat16),
        summat=sm.astype(ml_dtypes.bfloat16),
    )
    return per_core, weights, T_blk, T_tot, tile_start


def _build(T_blk, T_tot, tile_start):
    nc = bacc.Bacc(None, target_bir_lowering=False, num_devices=NCORES,
                   dynamic_dma_scratch_size=32768)

    dp = nc.declare_dram_parameter
    xT_aug = dp("xT_aug", [IN_DIM + 1, NPC], F32, isOutput=False)
    gsrc = dp("gsrc", [P, T_tot], I32, isOutput=False)
    onehot = dp("onehot", [P, T_tot * P], BF16, isOutput=False)
    ea_aug = dp("ea_aug", [T_tot * P, EDGE_DIM + 1], F32, isOutput=False)
    time_rows = dp("time_rows", [NBLK, TIME_DIM], F32, isOutput=False)
    W1_aug = dp("W1_aug", [IN_DIM + 1, HID], F32, isOutput=False)
    W_enc2 = dp("W_enc2", [HID, ENC_DIM], F32, isOutput=False)
    genc_col = dp("genc_col", [HID, 1], F32, isOutput=False)
    benc_col = dp("benc_col", [HID, 1], F32, isOutput=False)
    b_enc2_row = dp("b_enc2_row", [1, ENC_DIM], F32, isOutput=False)
    W_h_all = dp("W_h_all", [3, HID, HID], F32, isOutput=False)
    W_e_all = dp("W_e_all", [3, EDGE_DIM + 1, HID], F32, isOutput=False)
    ln_g_all = dp("ln_g_all", [3, HID], F32, isOutput=False)
    ln_b_all = dp("ln_b_all", [3, HID], F32, isOutput=False)
    W_a1 = dp("W_a1", [HID, P], F32, isOutput=False)
    b_a1_row = dp("b_a1_row", [1, P], F32, isOutput=False)
    W_a2_col = dp("W_a2_col", [P, 1], F32, isOutput=False)
    b_a2_in = dp("b_a2", [1, 1], F32, isOutput=False)
    W_g_row = dp("W_g_row", [1, HID], F32, isOutput=False)
    b_g_in = dp("b_g", [1, 1], F32, isOutput=False)
    typemask_in = dp("typemask", [P, NUM_TYPES], BF16, isOutput=False)
    summat_in = dp("summat", [P, 32], BF16, isOutput=False)
    out_dram = dp("out", [NBLK, HID], F32, isOutput=True)

    # bf16 shadows in DRAM (SWDGE cast once, HWDGE loads after)
    xT_bf = nc.dram_tensor("xT_bf", [IN_DIM + 1, NPC], BF16)
    ea_bf = nc.dram_tensor("ea_bf", [T_tot * P, EDGE_DIM + 1], BF16)
    W1_bf = nc.dram_tensor("W1_bf", [IN_DIM + 1, HID], BF16)
    Wh_bf = nc.dram_tensor("Wh_bf", [3, HID, HID], BF16)
    We_bf = nc.dram_tensor("We_bf", [3, EDGE_DIM + 1, HID], BF16)
    Wa1_bf = nc.dram_tensor("Wa1_bf", [HID, P], BF16)
    ba1_bf_d = nc.dram_tensor("ba1_bf", [1, P], BF16)
    Wa2_bf_d = nc.dram_tensor("Wa2_bf", [P, 1], BF16)
    ba2_bf_d = nc.dram_tensor("ba2_bf", [1, 1], BF16)
    tmr_bf_d = nc.dram_tensor("tmr_bf", [1, NBLK * TIME_DIM], BF16)

    h_locQ = [[nc.dram_tensor(f"h_locQ{i}_{qq}", [NPC // 4, HID], BF16)
               for qq in range(4)] for i in range(3)]
    h_all = [nc.dram_tensor(f"h_all{i}", [N, HID], BF16, addr_space="Shared")
             for i in range(3)]
    groups = [list(range(NCORES))]

    with tile.TileContext(nc) as tc:
        with (
            tc.tile_pool(name="big", bufs=1) as big,
            tc.tile_pool(name="acc", bufs=1) as accp,
            tc.tile_pool(name="gath", bufs=16) as gp,
            tc.tile_pool(name="work", bufs=4) as wk,
            tc.tile_pool(name="sm", bufs=8) as smp,
            tc.tile_pool(name="pp2", bufs=2, space="PSUM") as pp2,
            tc.tile_pool(name="pp1", bufs=1, space="PSUM") as pp1,
        ):
            # ---------- one-time casts (SWDGE) ----------
            nc.gpsimd.dma_start(out=xT_bf[:], in_=xT_aug[:])
            nc.gpsimd.dma_start(out=ea_bf[:], in_=ea_aug[:])
            nc.gpsimd.dma_start(out=W1_bf[:], in_=W1_aug[:])
            nc.gpsimd.dma_start(out=Wh_bf[:], in_=W_h_all[:])
            nc.gpsimd.dma_start(out=We_bf[:], in_=W_e_all[:])
            nc.gpsimd.dma_start(out=Wa1_bf[:], in_=W_a1[:])
            nc.gpsimd.dma_start(out=ba1_bf_d[:], in_=b_a1_row[:])
            nc.gpsimd.dma_start(out=Wa2_bf_d[:], in_=W_a2_col[:])
            nc.gpsimd.dma_start(out=ba2_bf_d[:], in_=b_a2_in[:])
            nc.gpsimd.dma_start(out=tmr_bf_d[:],
                                in_=time_rows[:].rearrange("b t -> (b t)")[None, :])

            # ---------- resident SBUF state ----------
            onehot_sb = big.tile([P, T_tot * P], BF16)
            nc.sync.dma_start(out=onehot_sb[:], in_=onehot[:])
            gsrc_sb = big.tile([P, T_tot], I32)
            nc.sync.dma_start(out=gsrc_sb[:], in_=gsrc[:])
            aggTe_sb = big.tile([EDGE_DIM + 1, NPC], BF16)
            W1_sb = big.tile([IN_DIM + 1, HID], BF16)
            nc.sync.dma_start(out=W1_sb[:], in_=W1_bf[:])
            Wh_sb = big.tile([P, 3, 2, HID], BF16)
            nc.sync.dma_start(out=Wh_sb[:],
                              in_=Wh_bf[:].rearrange("l (c p) n -> p l c n", p=P))
            We_sb = big.tile([EDGE_DIM + 1, 3, HID], BF16)
            nc.sync.dma_start(out=We_sb[:],
                              in_=We_bf[:].rearrange("l e n -> e l n"))
            Wa1_sb = big.tile([P, 2, P], BF16)
            nc.sync.dma_start(out=Wa1_sb[:],
                              in_=Wa1_bf[:].rearrange("(c p) a -> p c a", p=P))
            ba1_sb = big.tile([1, P], BF16)
            nc.sync.dma_start(out=ba1_sb[:], in_=ba1_bf_d[:])
            Wa2_sb = big.tile([P, 1], BF16)
            nc.sync.dma_start(out=Wa2_sb[:], in_=Wa2_bf_d[:])
            ba2_sb = big.tile([1, 1], BF16)
            nc.sync.dma_start(out=ba2_sb[:], in_=ba2_bf_d[:])
            time_sb = big.tile([1, NBLK * TIME_DIM], BF16)
            nc.sync.dma_start(out=time_sb[:], in_=tmr_bf_d[:])
            typemask_sb = big.tile([P, NUM_TYPES], BF16)
            nc.sync.dma_start(out=typemask_sb[:], in_=typemask_in[:])
            summat_sb = big.tile([P, 32], BF16)
            nc.sync.dma_start(out=summat_sb[:], in_=summat_in[:])
            # broadcast loads (SWDGE cast f32->bf16, partition step 0)
            lng_sb = big.tile([P, 3, HID], BF16)
            lnb_sb = big.tile([P, 3, HID], BF16)
            for i in range(3):
                nc.gpsimd.dma_start(out=lng_sb[:, i, :], in_=_bcast_rows(ln_g_all[i:i + 1, :]))
                nc.gpsimd.dma_start(out=lnb_sb[:, i, :], in_=_bcast_rows(ln_b_all[i:i + 1, :]))
            wg_sb = big.tile([P, HID], BF16)
            nc.gpsimd.dma_start(out=wg_sb[:], in_=_bcast_rows(W_g_row[:]))
            bg_sb = big.tile([P, 1], F32)
            nc.gpsimd.dma_start(out=bg_sb[:], in_=_bcast_rows(b_g_in[:]))

            eps_sb = big.tile([P, 1], F32)
            nc.vector.memset(eps_sb[:], EPS)
            ones_bf = big.tile([1, P], BF16)
            nc.vector.memset(ones_bf[:], 1.0)
            ones11_bf = big.tile([1, 1], BF16)
            nc.vector.memset(ones11_bf[:], 1.0)
            ones11_f = big.tile([1, 1], F32)
            nc.vector.memset(ones11_f[:], 1.0)
            ident = big.tile([P, P], BF16)
            make_identity(nc, ident[:])
            identf = big.tile([P, P], F32)
            make_identity(nc, identf[:])

            # ---------- encoder weight folds (device, f32) ----------
            W2_sb = big.tile([P, 2, ENC_DIM], F32)
            nc.sync.dma_start(out=W2_sb[:], in_=W_enc2[:].rearrange("(c p) n -> p c n", p=P))
            b2r_sb = big.tile([1, ENC_DIM], F32)
            nc.sync.dma_start(out=b2r_sb[:], in_=b_enc2_row[:])
            W2p_sb = big.tile([P, 2, ENC_DIM], BF16)
            b2p_sb = big.tile([1, ENC_DIM], BF16)
            ps_b2 = pp1.tile([1, ENC_DIM], F32, tag="psc")
            for k2 in range(2):
                gch = smp.tile([P, 1], F32, tag="gch")
                nc.sync.dma_start(out=gch[:], in_=genc_col[k2 * P:(k2 + 1) * P, :])
                nc.vector.tensor_scalar(out=W2p_sb[:, k2, :], in0=W2_sb[:, k2, :],
                                        scalar1=gch[:], scalar2=None, op0=OP.mult)
                bch = smp.tile([P, 1], F32, tag="bch")
                nc.sync.dma_start(out=bch[:], in_=benc_col[k2 * P:(k2 + 1) * P, :])
                nc.tensor.matmul(ps_b2[:], lhsT=bch[:], rhs=W2_sb[:, k2, :],
                                 start=(k2 == 0), stop=False)
            nc.tensor.matmul(ps_b2[:], lhsT=ones11_f[:], rhs=b2r_sb[:],
                             start=False, stop=True)
            nc.scalar.copy(b2p_sb[:], ps_b2[:])

            accs = [accp.tile([P, HID], BF16, tag=f"a{b}", name=f"acc{b}")
                    for b in range(NBLK)]

            # ---------- encoder ----------
            for b in range(NBLK):
                xt = wk.tile([IN_DIM + 1, P], BF16, tag="xt")
                nc.sync.dma_start(out=xt[:], in_=xT_bf[:, b * P:(b + 1) * P])
                ps_h1 = pp2.tile([P, HID], F32, tag="pa1")
                nc.tensor.matmul(ps_h1[:], lhsT=xt[:], rhs=W1_sb[:], start=True, stop=True)
                h1 = wk.tile([P, HID], F32, tag="h1")
                nc.scalar.activation(out=h1[:], in_=ps_h1[:], func=AF.Relu)
                stats = smp.tile([P, 6], F32, tag="st")
                nc.vector.bn_stats(out=stats[:], in_=h1[:])
                mv = smp.tile([P, 2], F32, tag="mv")
                nc.vector.bn_aggr(out=mv[:], in_=stats[:])
                rstd = smp.tile([P, 1], F32, tag="rstd")
                nc.scalar.activation(out=rstd[:], in_=mv[:, 1:2], func=AF.Sqrt,
                                     bias=eps_sb[:], scale=1.0)
                nc.vector.reciprocal(rstd[:], rstd[:])
                nm = smp.tile([P, 1], F32, tag="nm")
                nc.vector.tensor_scalar(out=nm[:], in0=mv[:, 0:1], scalar1=rstd[:],
                                        scalar2=-1.0, op0=OP.mult, op1=OP.mult)
                h1n0 = wk.tile([P, HID], BF16, tag="h1n0")
                nc.vector.tensor_scalar(out=h1n0[:], in0=h1[:], scalar1=rstd[:],
                                        scalar2=nm[:], op0=OP.mult, op1=OP.add)
                ps_tt = pp2.tile([P, HID], BF16, tag="pa0")
                nc.tensor.transpose(ps_tt[:, 0:P], h1n0[:, 0:P], ident[:])
                nc.tensor.transpose(ps_tt[:, P:HID], h1n0[:, P:HID], ident[:])
                tT = wk.tile([P, HID], BF16, tag="tT")
                nc.scalar.copy(tT[:], ps_tt[:])
                ps_he = pp2.tile([P, HID], F32, tag="pz")
                nc.tensor.matmul(ps_he[:, :ENC_DIM], lhsT=tT[:, 0:P], rhs=W2p_sb[:, 0, :],
                                 start=True, stop=False)
                nc.tensor.matmul(ps_he[:, :ENC_DIM], lhsT=tT[:, P:HID], rhs=W2p_sb[:, 1, :],
                                 start=False, stop=False)
                nc.tensor.matmul(ps_he[:, :ENC_DIM], lhsT=ones_bf[:], rhs=b2p_sb[:],
                                 start=False, stop=True)
                nc.tensor.matmul(ps_he[:, ENC_DIM:], lhsT=ones_bf[:],
                                 rhs=time_sb[:, b * TIME_DIM:(b + 1) * TIME_DIM],
                                 start=True, stop=True)
                henc = wk.tile([P, HID], BF16, tag="henc")
                nc.vector.tensor_copy(henc[:], ps_he[:])
                qq, bq = b // (NBLK // 4), b % (NBLK // 4)
                nc.sync.dma_start(out=h_locQ[0][qq][bq * P:(bq + 1) * P, :], in_=henc[:])
                if bq == NBLK // 4 - 1 and qq < 3:
                    nc.gpsimd.collective_compute(
                        "AllGather", OP.bypass, replica_groups=groups,
                        ins=[h_locQ[0][qq][:]],
                        outs=[h_all[0][qq * (N // 4):(qq + 1) * (N // 4), :]])

            # ---------- layers ----------
            score_all = big.tile([P, NBLK], F32)
            for i in range(NUM_LAYERS):
                nc.gpsimd.collective_compute(
                    "AllGather", OP.bypass, replica_groups=groups,
                    ins=[h_locQ[i][3][:]], outs=[h_all[i][3 * (N // 4):N, :]])
                if i == 0:
                    # aggTe pass (layer-independent) — fills the AG idle window
                    for b in range(NBLK):
                        ps_e = pp1.tile([EDGE_DIM + 1, P], F32, tag="pe")
                        t0, t1 = int(tile_start[b]), int(tile_start[b + 1])
                        for t in range(t0, t1):
                            ea_t = wk.tile([P, EDGE_DIM + 1], BF16, tag="ea")
                            nc.sync.dma_start(out=ea_t[:], in_=ea_bf[t * P:(t + 1) * P, :])
                            nc.tensor.matmul(ps_e[:], lhsT=ea_t[:],
                                             rhs=onehot_sb[:, t * P:(t + 1) * P],
                                             start=(t == t0), stop=(t == t1 - 1))
                        nc.vector.tensor_copy(aggTe_sb[:, b * P:(b + 1) * P], ps_e[:])

                for b in range(NBLK):
                    t0, t1 = int(tile_start[b]), int(tile_start[b + 1])
                    ps_a0 = pp2.tile([P, P], F32, tag="pa0")
                    ps_a1 = pp2.tile([P, P], F32, tag="pa1")
                    for t in range(t0, t1):
                        g = gp.tile([P, HID], BF16, tag="g")
                        nc.gpsimd.indirect_dma_start(
                            out=g[:], out_offset=None, in_=h_all[i][:],
                            in_offset=bass.IndirectOffsetOnAxis(
                                ap=gsrc_sb[:, t:t + 1], axis=0))
                        oh = onehot_sb[:, t * P:(t + 1) * P]
                        nc.tensor.matmul(ps_a0[:], lhsT=g[:, 0:P], rhs=oh,
                                         start=(t == t0), stop=(t == t1 - 1))
                        nc.tensor.matmul(ps_a1[:], lhsT=g[:, P:HID], rhs=oh,
                                         start=(t == t0), stop=(t == t1 - 1))
                    a0 = wk.tile([P, HID], BF16, tag="a0")
                    nc.vector.tensor_copy(a0[:, 0:P], ps_a0[:])
                    nc.vector.tensor_copy(a0[:, P:HID], ps_a1[:])
                    ps_z = pp2.tile([P, HID], F32, tag="pz")
                    nc.tensor.matmul(ps_z[:], lhsT=a0[:, 0:P], rhs=Wh_sb[:, i, 0, :],
                                     start=True, stop=False)
                    nc.tensor.matmul(ps_z[:], lhsT=a0[:, P:HID], rhs=Wh_sb[:, i, 1, :],
                                     start=False, stop=False)
                    nc.tensor.matmul(ps_z[:], lhsT=aggTe_sb[:, b * P:(b + 1) * P],
                                     rhs=We_sb[:, i, :], start=False, stop=True)
                    stats = smp.tile([P, 6], F32, tag="st")
                    nc.vector.bn_stats(out=stats[:], in_=ps_z[:])
                    mv = smp.tile([P, 2], F32, tag="mv")
                    nc.vector.bn_aggr(out=mv[:], in_=stats[:])
                    rstd = smp.tile([P, 1], F32, tag="rstd")
                    nc.scalar.activation(out=rstd[:], in_=mv[:, 1:2], func=AF.Sqrt,
                                         bias=eps_sb[:], scale=1.0)
                    nc.vector.reciprocal(rstd[:], rstd[:])
                    nm = smp.tile([P, 1], F32, tag="nm")
                    nc.vector.tensor_scalar(out=nm[:], in0=mv[:, 0:1], scalar1=rstd[:],
                                            scalar2=-1.0, op0=OP.mult, op1=OP.mult)
                    tnorm = wk.tile([P, HID], BF16, tag="tn")
                    nc.scalar.activation(out=tnorm[:], in_=ps_z[:], func=AF.Identity,
                                         bias=nm[:], scale=rstd[:])
                    u = wk.tile([P, HID], BF16, tag="u")
                    nc.vector.tensor_tensor(out=u[:], in0=tnorm[:], in1=lng_sb[:, i, :],
                                            op=OP.mult)
                    v = wk.tile([P, HID], BF16, tag="v")
                    nc.vector.tensor_tensor(out=v[:], in0=u[:], in1=lnb_sb[:, i, :],
                                            op=OP.add)
                    if i == 0:
                        nc.scalar.activation(out=accs[b][:], in_=v[:], func=AF.Silu)
                    else:
                        hn = wk.tile([P, HID], BF16, tag="hn")
                        nc.scalar.activation(out=hn[:], in_=v[:], func=AF.Silu)
                        nc.vector.tensor_tensor(out=accs[b][:], in0=accs[b][:],
                                                in1=hn[:], op=OP.add)
                    if i < 2:
                        qq, bq = b // (NBLK // 4), b % (NBLK // 4)
                        nc.sync.dma_start(out=h_locQ[i + 1][qq][bq * P:(bq + 1) * P, :],
                                          in_=accs[b][:])
                        if bq == NBLK // 4 - 1 and qq < 3:
                            nc.gpsimd.collective_compute(
                                "AllGather", OP.bypass, replica_groups=groups,
                                ins=[h_locQ[i + 1][qq][:]],
                                outs=[h_all[i + 1][qq * (N // 4):(qq + 1) * (N // 4), :]])
                    else:
                        # attention score for this block, overlapped with the
                        # remaining layer-2 gathers
                        ps_tt = pp2.tile([P, HID], BF16, tag="pa0")
                        nc.tensor.transpose(ps_tt[:, 0:P], accs[b][:, 0:P], ident[:])
                        nc.tensor.transpose(ps_tt[:, P:HID], accs[b][:, P:HID], ident[:])
                        hT = wk.tile([P, HID], BF16, tag="tT")
                        nc.scalar.copy(hT[:], ps_tt[:])
                        ps_s1 = pp2.tile([P, P], F32, tag="pz")
                        nc.tensor.matmul(ps_s1[:], lhsT=Wa1_sb[:, 0, :], rhs=hT[:, 0:P],
                                         start=True, stop=False)
                        nc.tensor.matmul(ps_s1[:], lhsT=Wa1_sb[:, 1, :], rhs=hT[:, P:HID],
                                         start=False, stop=False)
                        nc.tensor.matmul(ps_s1[:], lhsT=ba1_sb[:], rhs=ones_bf[:],
                                         start=False, stop=True)
                        s1t = wk.tile([P, P], BF16, tag="s1t")
                        nc.scalar.activation(out=s1t[:], in_=ps_s1[:], func=AF.Tanh)
                        ps_sc = pp1.tile([P, 1], F32, tag="psc")
                        nc.tensor.matmul(ps_sc[:], lhsT=s1t[:], rhs=Wa2_sb[:],
                                         start=True, stop=False)
                        nc.tensor.matmul(ps_sc[:], lhsT=ones_bf[:], rhs=ba2_sb[:],
                                         start=False, stop=True)
                        nc.vector.tensor_copy(score_all[:, b:b + 1], ps_sc[:])

            # batched softmax over (tile, type) groups
            ps_scT = pp2.tile([NBLK, P], F32, tag="pz")
            nc.tensor.transpose(ps_scT[:], score_all[:], identf[:])
            scT = big.tile([NBLK, P], F32)
            nc.vector.tensor_copy(scT[:], ps_scT[:])
            mx = big.tile([NBLK, NUM_TYPES], F32)
            nc.vector.tensor_reduce(out=mx[:], in_=scT[:].rearrange("p (k t) -> p t k", t=4),
                                    axis=mybir.AxisListType.X, op=OP.max)
            exr = big.tile([NBLK, P], F32)
            nc.vector.tensor_tensor(out=exr[:].rearrange("p (k t) -> p t k", t=4),
                                    in0=scT[:].rearrange("p (k t) -> p t k", t=4),
                                    in1=mx[:].to_broadcast([NBLK, NUM_TYPES, 32]),
                                    op=OP.subtract)
            nc.scalar.activation(out=exr[:], in_=exr[:], func=AF.Exp)
            den = big.tile([NBLK, NUM_TYPES], F32)
            nc.vector.tensor_reduce(out=den[:], in_=exr[:].rearrange("p (k t) -> p t k", t=4),
                                    axis=mybir.AxisListType.X, op=OP.add)
            nc.vector.reciprocal(den[:], den[:])
            attnT = big.tile([NBLK, P], F32)
            nc.vector.tensor_tensor(out=attnT[:].rearrange("p (k t) -> p t k", t=4),
                                    in0=exr[:].rearrange("p (k t) -> p t k", t=4),
                                    in1=den[:].to_broadcast([NBLK, NUM_TYPES, 32]),
                                    op=OP.mult)
            ps_at = pp2.tile([P, NBLK], F32, tag="pz")
            nc.tensor.transpose(ps_at[:], attnT[:], identf[0:NBLK, 0:NBLK])
            attn = big.tile([P, NBLK], F32)
            nc.vector.tensor_copy(attn[:], ps_at[:])

            pooled = [big.tile([P, HID], BF16, name=f"pooled{k}") for k in range(2)]
            for b in range(NBLK):
                wm = smp.tile([P, NUM_TYPES], BF16, tag="wm")
                nc.vector.tensor_scalar(out=wm[:], in0=typemask_sb[:],
                                        scalar1=attn[:, b:b + 1], scalar2=None,
                                        op0=OP.mult)
                ps_p = pp1.tile([NUM_TYPES, HID], F32, tag="pe")
                nc.tensor.matmul(ps_p[:], lhsT=wm[:], rhs=accs[b][:], start=True, stop=True)
                pstg = smp.tile([NUM_TYPES, HID], BF16, tag="pstg")
                nc.vector.tensor_copy(pstg[:], ps_p[:])
                half, bl = b // 32, b % 32
                nc.sync.dma_start(out=pooled[half][bl * 4:(bl + 1) * 4, :], in_=pstg[:])

            for half in range(2):
                pa = pooled[half]
                glm = big.tile([P, HID], BF16)
                nc.vector.tensor_tensor(out=glm[:], in0=pa[:], in1=wg_sb[:], op=OP.mult)
                gl = big.tile([P, 1], F32)
                nc.vector.tensor_reduce(out=gl[:], in_=glm[:],
                                        axis=mybir.AxisListType.X, op=OP.add)
                nc.vector.tensor_scalar(out=gl[:], in0=gl[:], scalar1=bg_sb[:],
                                        scalar2=None, op0=OP.add)
                ps_gr = pp1.tile([1, P], F32, tag="psc")
                nc.tensor.transpose(ps_gr[:], gl[:], identf[:])
                glT = big.tile([1, P], F32)
                nc.vector.tensor_copy(glT[:], ps_gr[:])
                mxg = big.tile([1, 32], F32)
                nc.vector.tensor_reduce(out=mxg[:], in_=glT[:].rearrange("p (b t) -> p b t", t=4),
                                        axis=mybir.AxisListType.X, op=OP.max)
                exg = big.tile([1, P], F32)
                nc.vector.tensor_tensor(out=exg[:].rearrange("p (b t) -> p b t", t=4),
                                        in0=glT[:].rearrange("p (b t) -> p b t", t=4),
                                        in1=mxg[:].to_broadcast([1, 32, 4]),
                                        op=OP.subtract)
                nc.scalar.activation(out=exg[:], in_=exg[:], func=AF.Exp)
                deng = big.tile([1, 32], F32)
                nc.vector.tensor_reduce(out=deng[:], in_=exg[:].rearrange("p (b t) -> p b t", t=4),
                                        axis=mybir.AxisListType.X, op=OP.add)
                nc.vector.reciprocal(deng[:], deng[:])
                gates_r = big.tile([1, P], BF16)
                nc.vector.tensor_tensor(out=gates_r[:].rearrange("p (b t) -> p b t", t=4),
                                        in0=exg[:].rearrange("p (b t) -> p b t", t=4),
                                        in1=deng[:].to_broadcast([1, 32, 4]),
                                        op=OP.mult)
                ps_gc = pp1.tile([P, 1], F32, tag="psc")
                nc.tensor.matmul(ps_gc[:], lhsT=gates_r[:], rhs=ones11_bf[:],
                                 start=True, stop=True)
                gc = big.tile([P, 1], F32)
                nc.vector.tensor_copy(gc[:], ps_gc[:])
                psc_sb = big.tile([P, HID], BF16)
                nc.vector.tensor_scalar(out=psc_sb[:], in0=pa[:], scalar1=gc[:],
                                        scalar2=None, op0=OP.mult)
                ps_o = pp1.tile([32, HID], F32, tag="pe")
                nc.tensor.matmul(ps_o[:], lhsT=summat_sb[:], rhs=psc_sb[:],
                                 start=True, stop=True)
                out_sb = big.tile([32, HID], F32)
                nc.vector.tensor_copy(out_sb[:], ps_o[:])
                nc.sync.dma_start(out=out_dram[half * 32:(half + 1) * 32, :], in_=out_sb[:])

    nc.compile()
    return nc


def kernel(**inputs):
    per_core, weights, T_blk, T_tot, tile_start = _prep(**inputs)
    key = (T_tot, tuple(int(t) for t in T_blk))
    if key not in _PROG_CACHE:
        _PROG_CACHE[key] = _build(T_blk, T_tot, tile_start)
    nc = _PROG_CACHE[key]
    in_maps = []
    for c in range(NCORES):
        m = dict(weights)
        pc = per_core[c]
        m.update(xT_aug=pc["xT_aug"], gsrc=pc["gsrc"], onehot=pc["onehot"],
                 ea_aug=pc["ea_aug"], time_rows=pc["time_rows"])
        in_maps.append(m)
    res = run_bass_kernel_spmd(nc, in_maps, list(range(NCORES)), trace=False)
    out = np.concatenate(
        [res.results[c]["out"].reshape(NBLK // T, T, HID) for c in range(NCORES)],
        axis=0)
    return np.ascontiguousarray(out.astype(np.float32))
